# revision 15
# baseline (speedup 1.0000x reference)
"""DeeperGCN Trainium2 kernel (8 NeuronCores, SPMD).

Strategy:
  - Nodes padded to NPAD = 8*NSH and sharded by dst across 8 cores.
  - Per conv layer, per-node messages u = m*exp(t*m), w = exp(t*m)
    (m = relu(h)+eps) are computed shard-wise (feature-major), transposed to
    node-major and AllGathered in FOUR row-quarters (double-buffered across
    layers) so edge gathers can start as soon as their quarter's table lands.
  - Edge aggregation runs as 4 passes (one per src quarter): dma_gather of
    uv rows by src (int16 indices, one SWDGE queue per pass), then a
    segmented sum over dst via TensorE matmuls with one-hot fp8 R matrices
    (host-precomputed, streamed); per-pass partial sums accumulate into a
    bf16 numden master (copy on pass 0, DVE add after).
  - MLP+LN evaluated feature-major with LN mean-centering and biases FOLDED
    into the weights host-side (ones-row trick for b1c), variance via
    ones-matmul of the square, rstd via ACT Abs_reciprocal_sqrt, all
    elementwise in bf16 at quarter-shard width.
Host preprocessing (edge bucketing, R matrices, transpose/cast of x,
weight centering) is numpy; only device NEFF time counts.
"""
import numpy as np
import ml_dtypes
from dataclasses import dataclass

EPS_MSG = 1e-7
LN_EPS = 1e-5
NCORES = 8


@dataclass
class Cfg:
    N: int = 100000
    E: int = 1000000
    F_IN: int = 500
    H: int = 64
    C: int = 3
    L: int = 3
    NSH: int = 12544            # nodes/core, multiple of 128
    NSW: int = 4                # src quarters (one AllGather + GS pass each)
    SLOTS_G: int = 32           # gather-buffer chunk slots per run
    MCH: int = 448              # MLP node-chunk (<=512)

    @property
    def NPAD(self):
        return NCORES * self.NSH

    @property
    def NW(self):
        return self.NSH // 128

    @property
    def WSZ(self):
        w = self.NPAD // self.NSW
        assert w <= 32768
        return w

    @property
    def HH(self):
        return 2 * self.H

    @property
    def FPAD(self):
        return ((self.F_IN + 127) // 128) * 128

    @property
    def NQ(self):
        return self.NSH // 4        # M/U-phase quarter width (3136)


CFG = Cfg()


def _cdiv(a, b):
    return (a + b - 1) // b


# --------------------------------------------------------------------------
# host: edge structures
# --------------------------------------------------------------------------

def build_edge_structs(edge_index, cfg=CFG):
    src = np.asarray(edge_index[0], np.int64)
    dst = np.asarray(edge_index[1], np.int64)
    NSH, NW, NSW, WSZ = cfg.NSH, cfg.NW, cfg.NSW, cfg.WSZ
    core = dst // NSH
    dstloc = dst % NSH
    w = dstloc // 128
    dcol = dstloc % 128
    s = src // WSZ
    srcloc = src % WSZ
    assert srcloc.max() < 32768

    counts = np.zeros((NCORES, NW, NSW), np.int64)
    np.add.at(counts, (core, w, s), 1)
    PBc = _cdiv(counts, 128).max(axis=0)           # [NW, NSW]
    PBc = np.maximum(PBc, 1)

    # per-si window runs with <= SLOTS_G chunks each
    runs = []
    for si in range(NSW):
        r, w0, acc = [], 0, 0
        for wi in range(NW):
            c = int(PBc[wi, si])
            if acc and acc + c > cfg.SLOTS_G:
                r.append((w0, wi, acc))
                w0, acc = wi, 0
            acc += c
        r.append((w0, NW, acc))
        runs.append(r)

    IW = [int(PBc[:, si].sum()) * 128 for si in range(NSW)]
    CTOT = int(PBc.sum())
    IW_base = np.concatenate([[0], np.cumsum(IW)]).astype(np.int64)

    gx_off = np.zeros((NW, NSW), np.int64)
    for si in range(NSW):
        o = 0
        for wi in range(NW):
            gx_off[wi, si] = o
            o += int(PBc[wi, si]) * 128
        assert o == IW[si]
    # R chunk id == global gather slot index
    ch_off = (IW_base[:-1][None, :] + gx_off) // 128   # [NW, NSW]

    order = np.lexsort((w, s, core))
    src_s, core_s, s_s, w_s, dcol_s = (
        srcloc[order], core[order], s[order], w[order], dcol[order])

    gidx_all, rmat_all = [], []
    for k in range(NCORES):
        sel = core_s == k
        ks, kw, kdc, ksrc = s_s[sel], w_s[sel], dcol_s[sel], src_s[sel]
        gidx = [np.zeros(IW[si], np.int16) for si in range(NSW)]
        rmat = np.zeros((128, CTOT, 128), ml_dtypes.float8_e4m3)
        for si in range(NSW):
            insel = ks == si
            kwsi, kdcsi, ksrcsi = kw[insel], kdc[insel], ksrc[insel]
            bnd = np.searchsorted(kwsi, np.arange(NW + 1))
            for wi in range(NW):
                a, b = bnd[wi], bnd[wi + 1]
                n = b - a
                cap = int(PBc[wi, si]) * 128
                assert n <= cap
                o = gx_off[wi, si]
                gidx[si][o: o + n] = ksrcsi[a:b].astype(np.int16)
                j = np.arange(n)
                rmat[j % 128, ch_off[wi, si] + j // 128, kdcsi[a:b]] = 1.0
        gidx_all.append(np.concatenate(gidx))
        rmat_all.append(rmat)

    return dict(PBc=PBc, runs=runs, IW=IW, CTOT=CTOT,
                gx_off=gx_off, IW_base=IW_base,
                gidx=gidx_all, rmat=rmat_all)


# --------------------------------------------------------------------------
# device builder
# --------------------------------------------------------------------------

def build_nc(structs, cfg=CFG):
    import concourse.bass as bass
    import concourse.tile as tile
    from concourse import bacc, mybir
    from contextlib import ExitStack

    dt = mybir.dt
    AF = mybir.ActivationFunctionType
    AL = mybir.AluOpType
    PBc, runs, IW, CTOT = (structs["PBc"], structs["runs"],
                           structs["IW"], structs["CTOT"])
    gx_off, IW_base = structs["gx_off"], structs["IW_base"]
    NSH, NW, NSW, WSZ = cfg.NSH, cfg.NW, cfg.NSW, cfg.WSZ
    H, HH, C, L, FPAD = cfg.H, cfg.HH, cfg.C, cfg.L, cfg.FPAD
    MCH, NQ = cfg.MCH, cfg.NQ
    NMC = NSH // MCH
    assert NMC * MCH == NSH
    CPQ = NQ // MCH
    assert CPQ * MCH == NQ
    IWALL = sum(IW)

    PCOL = {"encb": 0, "b2r": 1, "eps": 2, "tiny": 3, "zero": 4}
    nc_col = 5
    for l in range(L):
        for nm in ("g1", "be1", "b2c", "ng", "nb", "t"):
            PCOL[(nm, l)] = nc_col
            nc_col += 1
    NPCOL = nc_col

    nc = bacc.Bacc("TRN2", num_swdge_queues=4, dynamic_dma_scratch_size=32768)
    xt_d = nc.declare_dram_parameter("xt", [FPAD, NSH], dt.bfloat16, isOutput=False)
    encw_d = nc.declare_dram_parameter("encw", [128, FPAD // 128, H], dt.bfloat16, isOutput=False)
    w1_d = nc.declare_dram_parameter("w1", [H + 1, L, HH], dt.bfloat16, isOutput=False)
    w2_d = nc.declare_dram_parameter("w2", [HH, L, H], dt.bfloat16, isOutput=False)
    w2r_d = nc.declare_dram_parameter("w2r", [HH, H], dt.bfloat16, isOutput=False)
    linw_d = nc.declare_dram_parameter("linw", [H + 1, C], dt.bfloat16, isOutput=False)
    ident_d = nc.declare_dram_parameter("ident", [128, 128], dt.bfloat16, isOutput=False)
    params_d = nc.declare_dram_parameter("params", [128, NPCOL], dt.float32, isOutput=False)
    gidx_d = nc.declare_dram_parameter("gidx", [128, IWALL // 16], dt.int16, isOutput=False)
    rmat_d = nc.declare_dram_parameter("rmat", [128, CTOT, 128], dt.float8e4, isOutput=False)
    outp_d = nc.declare_dram_parameter("outp", [C, NSH], dt.float32, isOutput=True)

    uvshard = nc.dram_tensor("uvshard", [NSH, HH], dt.bfloat16)
    tabs = [nc.dram_tensor(f"uvtab{i}", [cfg.NPAD, HH], dt.bfloat16,
                           addr_space="Shared") for i in range(2)]

    with tile.TileContext(nc) as tc, ExitStack() as ctx:
        const = ctx.enter_context(tc.tile_pool(name="const", bufs=1))
        sb_par = const.tile([128, NPCOL], dt.float32)
        nc.sync.dma_start(sb_par[:], params_d[:])
        sb_encw = const.tile([128, FPAD // 128, H], dt.bfloat16)
        nc.sync.dma_start(sb_encw[:], encw_d[:])
        sb_w1 = const.tile([H + 1, L, HH], dt.bfloat16)
        nc.sync.dma_start(sb_w1[:], w1_d[:])
        sb_w2 = const.tile([HH, L, H], dt.bfloat16)
        nc.sync.dma_start(sb_w2[:], w2_d[:])
        sb_w2r = const.tile([HH, H], dt.bfloat16)
        nc.sync.dma_start(sb_w2r[:], w2r_d[:])
        sb_linw = const.tile([H + 1, C], dt.bfloat16)
        nc.sync.dma_start(sb_linw[:], linw_d[:])
        sb_id = const.tile([128, 128], dt.bfloat16)
        nc.sync.dma_start(sb_id[:], ident_d[:])
        sb_o128 = const.tile([128, 128], dt.bfloat16)
        nc.vector.memset(sb_o128[:], 1.0 / 128)
        sb_o64 = const.tile([H, H], dt.bfloat16)
        nc.vector.memset(sb_o64[:], 1.0 / H)
        sb_gidx = const.tile([128, IWALL // 16], dt.int16)
        nc.sync.dma_start(sb_gidx[:], gidx_d[:])

        def pcol(key, rows=128):
            cidx = PCOL[key]
            return sb_par[0:rows, cidx: cidx + 1]

        master = ctx.enter_context(tc.tile_pool(name="master", bufs=1))
        hT = master.tile([H + 1, NSH], dt.bfloat16)     # row H == 1.0
        numden = master.tile([HH, NSH], dt.bfloat16)
        uv2 = master.tile([128, NSH], dt.bfloat16)
        rbf65 = master.tile([H + 1, NQ], dt.bfloat16)   # row H == 1.0
        nc.vector.memset(hT[H:H + 1, :], 1.0)
        nc.vector.memset(rbf65[H:H + 1, :], 1.0)

        # ---------------- encoder: hT = (x @ enc_W + b)^T ----------------
        with tc.tile_pool(name="enc", bufs=3) as ep, \
             tc.tile_pool(name="encps", bufs=2, space="PSUM") as pp:
            for c in range(NMC):
                sl = slice(c * MCH, (c + 1) * MCH)
                xtile = ep.tile([128, FPAD // 128, MCH], dt.bfloat16)
                for fc in range(FPAD // 128):
                    nc.sync.dma_start(
                        xtile[:, fc, :], xt_d[fc * 128:(fc + 1) * 128, sl])
                ps = pp.tile([H, MCH], dt.float32)
                for fc in range(FPAD // 128):
                    nc.tensor.matmul(ps[:], sb_encw[:, fc, :], xtile[:, fc, :],
                                     start=(fc == 0), stop=(fc == FPAD // 128 - 1))
                nc.vector.tensor_scalar_add(hT[0:H, sl], ps[:], pcol("encb", H))

        # persistent pools for the conv pipeline
        t64 = ctx.enter_context(tc.tile_pool(name="t64", bufs=4))
        t128 = ctx.enter_context(tc.tile_pool(name="t128", bufs=4))
        utp = ctx.enter_context(tc.tile_pool(name="utp", bufs=1))
        stp = ctx.enter_context(tc.tile_pool(name="stp", bufs=3))
        gp = ctx.enter_context(tc.tile_pool(name="gp", bufs=2))
        rp = ctx.enter_context(tc.tile_pool(name="rp", bufs=2))
        tpp = ctx.enter_context(tc.tile_pool(name="tpp", bufs=2, space="PSUM"))
        gsp = ctx.enter_context(tc.tile_pool(name="gsp", bufs=2, space="PSUM"))
        mpa = ctx.enter_context(tc.tile_pool(name="mpa", bufs=2, space="PSUM"))
        mpb = ctx.enter_context(tc.tile_pool(name="mpb", bufs=2, space="PSUM"))

        # ---------------- conv layers ----------------
        conv_params = [0] + list(range(L))          # [0, 0, 1, 2]
        for conv, l in enumerate(conv_params):
            is_first = conv == 0
            tab = tabs[conv % 2]

            # U phase: uv2 rows 0:H = u = m*exp(t*m), rows H:HH = w
            for q in range(4):
                slq = slice(q * NQ, (q + 1) * NQ)
                m_t = utp.tile([H, NQ], dt.bfloat16, tag="mt")
                nc.vector.tensor_scalar(
                    m_t[:], hT[0:H, slq], 0.0, EPS_MSG,
                    AL.max, AL.add)
                wq = utp.tile([H, NQ], dt.bfloat16, tag="wq")
                nc.scalar.activation(wq[:], m_t[:], AF.Exp,
                                     scale=pcol(("t", l), H))
                # partition-crossing copy (tensor_scalar handles base shift)
                nc.vector.tensor_scalar_add(uv2[H:HH, slq], wq[:],
                                            pcol("zero", H))
                nc.vector.tensor_mul(uv2[0:H, slq], m_t[:], wq[:])
            # T phase: transpose to node-major -> uvshard, then AllGather
            for nt in range(NW):
                tp = tpp.tile([128, HH], dt.bfloat16, tag="tp")
                nc.tensor.transpose(
                    tp[:], uv2[:, nt * 128:(nt + 1) * 128], sb_id[:])
                st = stp.tile([128, HH], dt.bfloat16, tag="st")
                if nt % 2 == 0:
                    nc.scalar.copy(st[:], tp[:])
                else:
                    nc.vector.tensor_copy(st[:], tp[:])
                nc.sync.dma_start(uvshard[nt * 128:(nt + 1) * 128, :], st[:])
            nc.gpsimd.collective_compute(
                "AllGather", mybir.AluOpType.bypass,
                replica_groups=[list(range(NCORES))],
                ins=[uvshard[:, :]], outs=[tab[:, :]])

            # G+S: 4 passes, one per src quarter; accumulate into numden
            for si in range(NSW):
                for (w0, w1, slots) in runs[si]:
                    gbuf = gp.tile([128, slots, HH], dt.bfloat16, tag="gbuf")
                    a = int(IW_base[si] + gx_off[w0, si])
                    nidx = slots * 128
                    nc.gpsimd.dma_gather(
                        gbuf[:, 0:slots, :],
                        tab[si * WSZ: (si + 1) * WSZ, :],
                        sb_gidx[:, a // 16: (a + nidx) // 16],
                        nidx, nidx, HH, single_packet=False,
                        queue_num=si)
                    rtile = rp.tile([128, slots, 128], dt.float8e4, tag="rt")
                    nc.sync.dma_start(
                        rtile[:, 0:slots, :],
                        rmat_d[:, a // 128: a // 128 + slots, :])
                    for wi in range(w0, w1):
                        nch = int(PBc[wi, si])
                        base = int(gx_off[wi, si] - gx_off[w0, si]) // 128
                        ps = gsp.tile([HH, 128], dt.float32, tag="gs")
                        for j in range(nch):
                            nc.tensor.matmul(
                                ps[:], gbuf[:, base + j, :],
                                rtile[:, base + j, :],
                                start=(j == 0), stop=(j == nch - 1))
                        wsl = slice(wi * 128, (wi + 1) * 128)
                        if si == 0:
                            if wi % 2 == 0:
                                nc.scalar.copy(numden[:, wsl], ps[:])
                            else:
                                nc.vector.tensor_copy(numden[:, wsl], ps[:])
                        else:
                            nc.vector.tensor_add(numden[:, wsl],
                                                 numden[:, wsl], ps[:])

            # M phase: agg -> MLP(+folded LN) -> residual, per quarter shard
            for q in range(4):
                slq = slice(q * NQ, (q + 1) * NQ)
                dn_lo = t64.tile([H, NQ], dt.bfloat16, tag="t64")
                nc.vector.tensor_scalar_max(dn_lo[:], numden[H:HH, slq], 0.5)
                s_t = t64.tile([H, NQ], dt.bfloat16, tag="t64")
                nc.scalar.activation(s_t[:], dn_lo[:],
                                     AF.Abs_reciprocal_sqrt,
                                     bias=pcol("tiny", H))
                rden = t64.tile([H, NQ], dt.bfloat16, tag="t64")
                nc.vector.tensor_mul(rden[:], s_t[:], s_t[:])
                t1 = t64.tile([H, NQ], dt.bfloat16, tag="t64")
                nc.vector.tensor_mul(t1[:], numden[0:H, slq], rden[:])
                nc.vector.tensor_add(rbf65[0:H, :], t1[:], hT[0:H, slq])
                ycs = t128.tile([HH, NQ], dt.bfloat16, tag="t128")
                rstd = t128.tile([HH, NQ], dt.bfloat16, tag="t128")
                for c in range(CPQ):
                    lo = c * MCH
                    ps1 = mpa.tile([HH, MCH], dt.float32, tag="mma")
                    nc.tensor.matmul(ps1[:], sb_w1[:, l, :],
                                     rbf65[:, lo:lo + MCH])
                    nc.vector.tensor_copy(ycs[:, lo:lo + MCH], ps1[:])
                sq = t128.tile([HH, NQ], dt.bfloat16, tag="t128")
                nc.vector.tensor_mul(sq[:], ycs[:], ycs[:])
                for c in range(CPQ):
                    lo = c * MCH
                    pv = mpa.tile([HH, MCH], dt.float32, tag="mma")
                    nc.tensor.matmul(pv[:], sb_o128[:], sq[:, lo:lo + MCH])
                    nc.scalar.activation(rstd[:, lo:lo + MCH], pv[:],
                                         AF.Abs_reciprocal_sqrt,
                                         bias=pcol("eps"))
                t_t = t128.tile([HH, NQ], dt.bfloat16, tag="t128")
                nc.vector.tensor_mul(t_t[:], ycs[:], rstd[:])
                h1 = t128.tile([HH, NQ], dt.bfloat16, tag="t128")
                nc.scalar.activation(h1[:], t_t[:], AF.Relu,
                                     bias=pcol(("be1", l)),
                                     scale=pcol(("g1", l)))
                if is_first:
                    for c in range(CPQ):
                        lo = c * MCH
                        sl = slice(q * NQ + lo, q * NQ + lo + MCH)
                        ps2 = mpb.tile([H, MCH], dt.float32, tag="mmb")
                        nc.tensor.matmul(ps2[:], sb_w2r[:, :], h1[:, lo:lo + MCH])
                        nc.vector.tensor_scalar_add(hT[0:H, sl], ps2[:],
                                                    pcol("b2r", H))
                else:
                    ycs2 = t64.tile([H, NQ], dt.bfloat16, tag="t64")
                    rstd2 = t64.tile([H, NQ], dt.bfloat16, tag="t64")
                    for c in range(CPQ):
                        lo = c * MCH
                        ps2 = mpb.tile([H, MCH], dt.float32, tag="mmb")
                        nc.tensor.matmul(ps2[:], sb_w2[:, l, :], h1[:, lo:lo + MCH])
                        nc.vector.tensor_scalar_add(ycs2[:, lo:lo + MCH], ps2[:],
                                                    pcol(("b2c", l), H))
                    sq2 = t64.tile([H, NQ], dt.bfloat16, tag="t64")
                    nc.vector.tensor_mul(sq2[:], ycs2[:], ycs2[:])
                    for c in range(CPQ):
                        lo = c * MCH
                        pv2 = mpb.tile([H, MCH], dt.float32, tag="mmb")
                        nc.tensor.matmul(pv2[:], sb_o64[:], sq2[:, lo:lo + MCH])
                        nc.scalar.activation(rstd2[:, lo:lo + MCH], pv2[:],
                                             AF.Abs_reciprocal_sqrt,
                                             bias=pcol("eps", H))
                    t2 = t64.tile([H, NQ], dt.bfloat16, tag="t64")
                    nc.vector.tensor_mul(t2[:], ycs2[:], rstd2[:])
                    c_t = t64.tile([H, NQ], dt.bfloat16, tag="t64")
                    nc.scalar.activation(c_t[:], t2[:], AF.Relu,
                                         bias=pcol(("nb", l), H),
                                         scale=pcol(("ng", l), H))
                    nc.vector.tensor_add(hT[0:H, slq], hT[0:H, slq], c_t[:])

        # ---------------- final head (reuses conv pools) ----------------
        for q in range(4):
            slq = slice(q * NQ, (q + 1) * NQ)
            yc = t128.tile([H, NQ], dt.bfloat16, tag="t128")
            rstd = t64.tile([H, NQ], dt.bfloat16, tag="t64")
            for c in range(CPQ):
                lo = c * MCH
                sl = slice(q * NQ + lo, q * NQ + lo + MCH)
                pmu = mpb.tile([H, MCH], dt.float32, tag="mmb")
                nc.tensor.matmul(pmu[:], sb_o64[:], hT[0:H, sl])
                nc.vector.tensor_sub(yc[:, lo:lo + MCH], hT[0:H, sl], pmu[:])
            sq = t64.tile([H, NQ], dt.bfloat16, tag="t64")
            nc.vector.tensor_mul(sq[:], yc[:], yc[:])
            for c in range(CPQ):
                lo = c * MCH
                pv = mpb.tile([H, MCH], dt.float32, tag="mmb")
                nc.tensor.matmul(pv[:], sb_o64[:], sq[:, lo:lo + MCH])
                nc.scalar.activation(rstd[:, lo:lo + MCH], pv[:],
                                     AF.Abs_reciprocal_sqrt, bias=pcol("eps", H))
            t_t = t64.tile([H, NQ], dt.bfloat16, tag="t64")
            nc.vector.tensor_mul(t_t[:], yc[:], rstd[:])
            f65 = t128.tile([H + 1, NQ], dt.bfloat16, tag="t128")
            nc.vector.memset(f65[H:H + 1, :], 1.0)
            nc.scalar.activation(f65[0:H, :], t_t[:], AF.Relu,
                                 bias=pcol(("nb", 0), H),
                                 scale=pcol(("ng", 0), H))
            for c in range(CPQ):
                lo = c * MCH
                sl = slice(q * NQ + lo, q * NQ + lo + MCH)
                pso = mpa.tile([C, MCH], dt.float32, tag="mma")
                nc.tensor.matmul(pso[:], sb_linw[:, :], f65[:, lo:lo + MCH])
                ot = stp.tile([C, MCH], dt.float32, tag="ot")
                nc.vector.tensor_copy(ot[:], pso[:])
                nc.sync.dma_start(outp_d[:, sl], ot[:])

    nc.compile()
    return nc, NPCOL, PCOL


# --------------------------------------------------------------------------
# host: input packing
# --------------------------------------------------------------------------

def pack_inputs(inputs, structs, NPCOL, PCOL, cfg=CFG):
    bf16 = ml_dtypes.bfloat16
    NSH, NPAD, FPAD = cfg.NSH, cfg.NPAD, cfg.FPAD
    H, HH, C, L = cfg.H, cfg.HH, cfg.C, cfg.L

    x = np.asarray(inputs["x"], np.float32)
    xp = np.zeros((NPAD, FPAD), np.float32)
    xp[: x.shape[0], : x.shape[1]] = x

    encw = np.zeros((FPAD, H), np.float32)
    encw[: cfg.F_IN] = np.asarray(inputs["enc_W"], np.float32)
    encw = np.ascontiguousarray(
        encw.reshape(FPAD // 128, 128, H).transpose(1, 0, 2)).astype(bf16)

    # W1 with LN mean-centering folded in + centered bias as a 65th row
    W1 = np.asarray(inputs["W1"], np.float32)          # [L, H, HH]
    b1 = np.asarray(inputs["b1"], np.float32)          # [L, HH]
    W1c = W1 - W1.mean(axis=2, keepdims=True)
    b1c = b1 - b1.mean(axis=1, keepdims=True)
    w1s = np.concatenate([W1c, b1c[:, None, :]], axis=1)   # [L, H+1, HH]
    w1s = np.ascontiguousarray(w1s.transpose(1, 0, 2)).astype(bf16)

    W2 = np.asarray(inputs["W2"], np.float32)          # [L, HH, H]
    W2c = W2 - W2.mean(axis=2, keepdims=True)
    w2s = np.ascontiguousarray(W2c.transpose(1, 0, 2)).astype(bf16)
    w2r = np.ascontiguousarray(W2[0]).astype(bf16)     # uncentered, conv 0

    linw = np.asarray(inputs["lin_W"], np.float32)
    linb = np.asarray(inputs["lin_b"], np.float32)
    linw65 = np.concatenate([linw, linb[None, :]], axis=0).astype(bf16)
    ident = np.eye(128, dtype=bf16)

    b2 = np.asarray(inputs["b2"], np.float32)          # [L, H]
    b2c = b2 - b2.mean(axis=1, keepdims=True)

    params = np.zeros((128, NPCOL), np.float32)
    params[:H, PCOL["encb"]] = inputs["enc_b"]
    params[:H, PCOL["b2r"]] = b2[0]
    params[:, PCOL["eps"]] = LN_EPS
    params[:, PCOL["tiny"]] = 1e-30
    for l in range(L):
        params[:, PCOL[("g1", l)]] = inputs["g1"][l]
        params[:, PCOL[("be1", l)]] = inputs["be1"][l]
        params[:H, PCOL[("b2c", l)]] = b2c[l]
        params[:H, PCOL[("ng", l)]] = inputs["ng"][l]
        params[:H, PCOL[("nb", l)]] = inputs["nb"][l]
        params[:, PCOL[("t", l)]] = float(np.asarray(inputs["t"][l]))

    in_maps = []
    for k in range(NCORES):
        xs = np.ascontiguousarray(
            xp[k * NSH:(k + 1) * NSH].T).astype(bf16)
        gi = structs["gidx"][k]
        gw = np.tile(np.ascontiguousarray(gi.reshape(-1, 16).T), (8, 1))
        in_maps.append({
            "xt": xs, "encw": encw, "w1": w1s, "w2": w2s, "w2r": w2r,
            "linw": linw65, "ident": ident, "params": params, "gidx": gw,
            "rmat": structs["rmat"][k],
        })
    return in_maps


def _run(inputs, cfg=CFG, trace=False, tmpdir=None):
    import sys
    sys.path.insert(0, "/root/problem")
    from concourse.bass_utils import run_bass_kernel_spmd

    structs = build_edge_structs(inputs["edge_index"], cfg)
    nc, NPCOL, PCOL = build_nc(structs, cfg)
    in_maps = pack_inputs(inputs, structs, NPCOL, PCOL, cfg)
    res = run_bass_kernel_spmd(nc, in_maps, list(range(NCORES)), trace=trace,
                               tmpdir=tmpdir)
    outs = [res.results[k]["outp"] for k in range(NCORES)]  # [C, NSH] each
    full = np.concatenate(outs, axis=1).T                   # [NPAD, C]
    return np.ascontiguousarray(full[: cfg.N]).astype(np.float32), res


def kernel(**inputs) -> np.ndarray:
    out, _ = _run(inputs)
    return out


# revision 17
# speedup vs baseline: 1.6786x; 1.6786x over previous
"""DeeperGCN Trainium2 kernel (8 NeuronCores, SPMD).

Strategy:
  - Nodes padded to NPAD = 8*NSH and sharded by dst across 8 cores.
  - Per conv layer, per-node messages u = m*exp(t*m), w = exp(t*m)
    (m = relu(h)+eps) are computed shard-wise (feature-major), transposed to
    node-major and AllGathered in FOUR row-quarters (double-buffered across
    layers) so edge gathers can start as soon as their quarter's table lands.
  - Edge aggregation runs as 4 passes (one per src quarter): dma_gather of
    uv rows by src (int16 indices, one SWDGE queue per pass), then a
    segmented sum over dst via TensorE matmuls with one-hot fp8 R matrices
    (host-precomputed, streamed); per-pass partial sums accumulate into a
    bf16 numden master (copy on pass 0, DVE add after).
  - MLP+LN evaluated feature-major with LN mean-centering and biases FOLDED
    into the weights host-side (ones-row trick for b1c), variance via
    ones-matmul of the square, rstd via ACT Abs_reciprocal_sqrt, all
    elementwise in bf16 at quarter-shard width.
Host preprocessing (edge bucketing, R matrices, transpose/cast of x,
weight centering) is numpy; only device NEFF time counts.
"""
import numpy as np
import ml_dtypes
from dataclasses import dataclass

EPS_MSG = 1e-7
LN_EPS = 1e-5
NCORES = 8


@dataclass
class Cfg:
    N: int = 100000
    E: int = 1000000
    F_IN: int = 500
    H: int = 64
    C: int = 3
    L: int = 3
    NSH: int = 12544            # nodes/core, multiple of 128
    NSW: int = 4                # src quarters (one AllGather + GS pass each)
    SLOTS_G: int = 24           # gather-buffer chunk slots per run
    MCH: int = 448              # MLP node-chunk (<=512)

    @property
    def NPAD(self):
        return NCORES * self.NSH

    @property
    def NW(self):
        return self.NSH // 128

    @property
    def WSZ(self):
        w = self.NPAD // self.NSW
        assert w <= 32768
        return w

    @property
    def HH(self):
        return 2 * self.H

    @property
    def FPAD(self):
        return ((self.F_IN + 127) // 128) * 128

    @property
    def NQ(self):
        return self.NSH // 4        # M/U-phase quarter width (3136)


CFG = Cfg()


def _cdiv(a, b):
    return (a + b - 1) // b


# --------------------------------------------------------------------------
# host: edge structures
# --------------------------------------------------------------------------

def build_edge_structs(edge_index, cfg=CFG):
    src = np.asarray(edge_index[0], np.int64)
    dst = np.asarray(edge_index[1], np.int64)
    NSH, NW, NSW, WSZ = cfg.NSH, cfg.NW, cfg.NSW, cfg.WSZ
    core = dst // NSH
    dstloc = dst % NSH
    w = dstloc // 128
    dcol = dstloc % 128
    s = src // WSZ
    srcloc = src % WSZ
    assert srcloc.max() < 32768

    counts = np.zeros((NCORES, NW, NSW), np.int64)
    np.add.at(counts, (core, w, s), 1)
    PBc = _cdiv(counts, 128).max(axis=0)           # [NW, NSW]
    PBc = np.maximum(PBc, 1)

    # per-si window runs with <= SLOTS_G chunks each
    runs = []
    for si in range(NSW):
        r, w0, acc = [], 0, 0
        for wi in range(NW):
            c = int(PBc[wi, si])
            if acc and acc + c > cfg.SLOTS_G:
                r.append((w0, wi, acc))
                w0, acc = wi, 0
            acc += c
        r.append((w0, NW, acc))
        runs.append(r)

    IW = [int(PBc[:, si].sum()) * 128 for si in range(NSW)]
    CTOT = int(PBc.sum())
    IW_base = np.concatenate([[0], np.cumsum(IW)]).astype(np.int64)

    gx_off = np.zeros((NW, NSW), np.int64)
    for si in range(NSW):
        o = 0
        for wi in range(NW):
            gx_off[wi, si] = o
            o += int(PBc[wi, si]) * 128
        assert o == IW[si]
    # R chunk id == global gather slot index
    ch_off = (IW_base[:-1][None, :] + gx_off) // 128   # [NW, NSW]

    order = np.lexsort((w, s, core))
    src_s, core_s, s_s, w_s, dcol_s = (
        srcloc[order], core[order], s[order], w[order], dcol[order])

    gidx_all, rmat_all = [], []
    for k in range(NCORES):
        sel = core_s == k
        ks, kw, kdc, ksrc = s_s[sel], w_s[sel], dcol_s[sel], src_s[sel]
        gidx = [np.zeros(IW[si], np.int16) for si in range(NSW)]
        rmat = np.zeros((128, CTOT, 128), ml_dtypes.float8_e4m3)
        for si in range(NSW):
            insel = ks == si
            kwsi, kdcsi, ksrcsi = kw[insel], kdc[insel], ksrc[insel]
            bnd = np.searchsorted(kwsi, np.arange(NW + 1))
            for wi in range(NW):
                a, b = bnd[wi], bnd[wi + 1]
                n = b - a
                cap = int(PBc[wi, si]) * 128
                assert n <= cap
                o = gx_off[wi, si]
                gidx[si][o: o + n] = ksrcsi[a:b].astype(np.int16)
                j = np.arange(n)
                rmat[j % 128, ch_off[wi, si] + j // 128, kdcsi[a:b]] = 1.0
        gidx_all.append(np.concatenate(gidx))
        rmat_all.append(rmat)

    return dict(PBc=PBc, runs=runs, IW=IW, CTOT=CTOT,
                gx_off=gx_off, IW_base=IW_base,
                gidx=gidx_all, rmat=rmat_all)


# --------------------------------------------------------------------------
# device builder
# --------------------------------------------------------------------------

def build_nc(structs, cfg=CFG):
    import concourse.bass as bass
    import concourse.tile as tile
    from concourse import bacc, mybir
    from contextlib import ExitStack

    dt = mybir.dt
    AF = mybir.ActivationFunctionType
    AL = mybir.AluOpType
    PBc, runs, IW, CTOT = (structs["PBc"], structs["runs"],
                           structs["IW"], structs["CTOT"])
    gx_off, IW_base = structs["gx_off"], structs["IW_base"]
    NSH, NW, NSW, WSZ = cfg.NSH, cfg.NW, cfg.NSW, cfg.WSZ
    H, HH, C, L, FPAD = cfg.H, cfg.HH, cfg.C, cfg.L, cfg.FPAD
    MCH, NQ = cfg.MCH, cfg.NQ
    NMC = NSH // MCH
    assert NMC * MCH == NSH
    CPQ = NQ // MCH
    assert CPQ * MCH == NQ
    IWALL = sum(IW)

    PCOL = {"encb": 0, "b2r": 1, "eps": 2, "tiny": 3, "zero": 4}
    nc_col = 5
    for l in range(L):
        for nm in ("g1", "be1", "b2c", "ng", "nb", "t"):
            PCOL[(nm, l)] = nc_col
            nc_col += 1
    NPCOL = nc_col

    nc = bacc.Bacc("TRN2", num_swdge_queues=4, dynamic_dma_scratch_size=32768)
    xt_d = nc.declare_dram_parameter("xt", [FPAD, NSH], dt.bfloat16, isOutput=False)
    encw_d = nc.declare_dram_parameter("encw", [128, FPAD // 128, H], dt.bfloat16, isOutput=False)
    w1_d = nc.declare_dram_parameter("w1", [H + 1, L, HH], dt.bfloat16, isOutput=False)
    w2_d = nc.declare_dram_parameter("w2", [HH, L, H], dt.bfloat16, isOutput=False)
    w2r_d = nc.declare_dram_parameter("w2r", [HH, H], dt.bfloat16, isOutput=False)
    linw_d = nc.declare_dram_parameter("linw", [H + 1, C], dt.bfloat16, isOutput=False)
    ident_d = nc.declare_dram_parameter("ident", [128, 128], dt.bfloat16, isOutput=False)
    params_d = nc.declare_dram_parameter("params", [128, NPCOL], dt.float32, isOutput=False)
    gidx_d = nc.declare_dram_parameter("gidx", [128, IWALL // 16], dt.int16, isOutput=False)
    rmat_d = nc.declare_dram_parameter("rmat", [128, CTOT, 128], dt.float8e4, isOutput=False)
    outp_d = nc.declare_dram_parameter("outp", [C, NSH], dt.float32, isOutput=True)

    uvshard = nc.dram_tensor("uvshard", [NSH, HH], dt.bfloat16)
    tabs = [nc.dram_tensor(f"uvtab{i}", [cfg.NPAD, HH], dt.bfloat16,
                           addr_space="Shared") for i in range(2)]

    with tile.TileContext(nc) as tc, ExitStack() as ctx:
        const = ctx.enter_context(tc.tile_pool(name="const", bufs=1))
        sb_par = const.tile([128, NPCOL], dt.float32)
        nc.sync.dma_start(sb_par[:], params_d[:])
        sb_encw = const.tile([128, FPAD // 128, H], dt.bfloat16)
        nc.sync.dma_start(sb_encw[:], encw_d[:])
        sb_w1 = const.tile([H + 1, L, HH], dt.bfloat16)
        nc.sync.dma_start(sb_w1[:], w1_d[:])
        sb_w2 = const.tile([HH, L, H], dt.bfloat16)
        nc.sync.dma_start(sb_w2[:], w2_d[:])
        sb_w2r = const.tile([HH, H], dt.bfloat16)
        nc.sync.dma_start(sb_w2r[:], w2r_d[:])
        sb_linw = const.tile([H + 1, C], dt.bfloat16)
        nc.sync.dma_start(sb_linw[:], linw_d[:])
        sb_id = const.tile([128, 128], dt.bfloat16)
        nc.sync.dma_start(sb_id[:], ident_d[:])
        sb_o128 = const.tile([128, 128], dt.bfloat16)
        nc.vector.memset(sb_o128[:], 1.0 / 128)
        sb_o64 = const.tile([H, H], dt.bfloat16)
        nc.vector.memset(sb_o64[:], 1.0 / H)
        sb_gidx = const.tile([128, IWALL // 16], dt.int16)
        nc.sync.dma_start(sb_gidx[:], gidx_d[:])

        def pcol(key, rows=128):
            cidx = PCOL[key]
            return sb_par[0:rows, cidx: cidx + 1]

        master = ctx.enter_context(tc.tile_pool(name="master", bufs=1))
        hT = master.tile([H + 1, NSH], dt.bfloat16)     # row H == 1.0
        numden = master.tile([HH, NSH], dt.bfloat16)
        uv2 = master.tile([128, NSH // 2], dt.bfloat16)
        rbf65 = master.tile([H + 1, NQ], dt.bfloat16)   # row H == 1.0
        nc.vector.memset(hT[H:H + 1, :], 1.0)
        nc.vector.memset(rbf65[H:H + 1, :], 1.0)

        # ---------------- encoder: hT = (x @ enc_W + b)^T ----------------
        with tc.tile_pool(name="enc", bufs=3) as ep, \
             tc.tile_pool(name="encps", bufs=2, space="PSUM") as pp:
            for c in range(NMC):
                sl = slice(c * MCH, (c + 1) * MCH)
                xtile = ep.tile([128, FPAD // 128, MCH], dt.bfloat16)
                for fc in range(FPAD // 128):
                    nc.sync.dma_start(
                        xtile[:, fc, :], xt_d[fc * 128:(fc + 1) * 128, sl])
                ps = pp.tile([H, MCH], dt.float32)
                for fc in range(FPAD // 128):
                    nc.tensor.matmul(ps[:], sb_encw[:, fc, :], xtile[:, fc, :],
                                     start=(fc == 0), stop=(fc == FPAD // 128 - 1))
                nc.vector.tensor_scalar_add(hT[0:H, sl], ps[:], pcol("encb", H))

        # persistent pools for the conv pipeline
        t64 = ctx.enter_context(tc.tile_pool(name="t64", bufs=4))
        t128 = ctx.enter_context(tc.tile_pool(name="t128", bufs=4))
        utp = ctx.enter_context(tc.tile_pool(name="utp", bufs=1))
        stp = ctx.enter_context(tc.tile_pool(name="stp", bufs=3))
        gp = ctx.enter_context(tc.tile_pool(name="gp", bufs=4))
        rp = ctx.enter_context(tc.tile_pool(name="rp", bufs=4))
        tpp = ctx.enter_context(tc.tile_pool(name="tpp", bufs=2, space="PSUM"))
        gsp = ctx.enter_context(tc.tile_pool(name="gsp", bufs=2, space="PSUM"))
        mpa = ctx.enter_context(tc.tile_pool(name="mpa", bufs=2, space="PSUM"))
        mpb = ctx.enter_context(tc.tile_pool(name="mpb", bufs=2, space="PSUM"))

        # ---------------- conv layers ----------------
        conv_params = [0] + list(range(L))          # [0, 0, 1, 2]
        for conv, l in enumerate(conv_params):
            is_first = conv == 0
            tab = tabs[conv % 2]

            # U phase: uv2 rows 0:H = u = m*exp(t*m), rows H:HH = w
            # (half-shard staging buffer; transpose each half then reuse)
            NWH = NW // 2
            for half in range(2):
                for qq in range(2):
                    q = half * 2 + qq
                    slq = slice(q * NQ, (q + 1) * NQ)
                    sll = slice(qq * NQ, (qq + 1) * NQ)
                    m_t = utp.tile([H, NQ], dt.bfloat16, tag="mt")
                    nc.vector.tensor_scalar(
                        m_t[:], hT[0:H, slq], 0.0, EPS_MSG,
                        AL.max, AL.add)
                    wq = utp.tile([H, NQ], dt.bfloat16, tag="wq")
                    nc.scalar.activation(wq[:], m_t[:], AF.Exp,
                                         scale=pcol(("t", l), H))
                    # partition-crossing copy (tensor_scalar base shift)
                    nc.vector.tensor_scalar_add(uv2[H:HH, sll], wq[:],
                                                pcol("zero", H))
                    nc.vector.tensor_mul(uv2[0:H, sll], m_t[:], wq[:])
                for ntl in range(NWH):
                    nt = half * NWH + ntl
                    tp = tpp.tile([128, HH], dt.bfloat16, tag="tp")
                    nc.tensor.transpose(
                        tp[:], uv2[:, ntl * 128:(ntl + 1) * 128], sb_id[:])
                    st = stp.tile([128, HH], dt.bfloat16, tag="st")
                    if nt % 2 == 0:
                        nc.scalar.copy(st[:], tp[:])
                    else:
                        nc.vector.tensor_copy(st[:], tp[:])
                    nc.sync.dma_start(uvshard[nt * 128:(nt + 1) * 128, :],
                                      st[:])
            nc.gpsimd.collective_compute(
                "AllGather", mybir.AluOpType.bypass,
                replica_groups=[list(range(NCORES))],
                ins=[uvshard[:, :]], outs=[tab[:, :]])

            # G+S: 4 passes interleaved round-robin so all 4 SWDGE queues
            # drain concurrently; accumulate into numden
            nrun = max(len(r) for r in runs)
            ilv = [(si, runs[si][k]) for k in range(nrun)
                   for si in range(NSW) if k < len(runs[si])]
            for (si, (w0, w1, slots)) in ilv:
                if True:
                    gbuf = gp.tile([128, slots, HH], dt.bfloat16, tag="gbuf")
                    a = int(IW_base[si] + gx_off[w0, si])
                    nidx = slots * 128
                    nc.gpsimd.dma_gather(
                        gbuf[:, 0:slots, :],
                        tab[si * WSZ: (si + 1) * WSZ, :],
                        sb_gidx[:, a // 16: (a + nidx) // 16],
                        nidx, nidx, HH, single_packet=False,
                        queue_num=si)
                    rtile = rp.tile([128, slots, 128], dt.float8e4, tag="rt")
                    nc.sync.dma_start(
                        rtile[:, 0:slots, :],
                        rmat_d[:, a // 128: a // 128 + slots, :])
                    for wi in range(w0, w1):
                        nch = int(PBc[wi, si])
                        base = int(gx_off[wi, si] - gx_off[w0, si]) // 128
                        ps = gsp.tile([HH, 128], dt.float32, tag="gs")
                        for j in range(nch):
                            nc.tensor.matmul(
                                ps[:], gbuf[:, base + j, :],
                                rtile[:, base + j, :],
                                start=(j == 0), stop=(j == nch - 1))
                        wsl = slice(wi * 128, (wi + 1) * 128)
                        if si == 0:
                            if wi % 2 == 0:
                                nc.scalar.copy(numden[:, wsl], ps[:])
                            else:
                                nc.vector.tensor_copy(numden[:, wsl], ps[:])
                        else:
                            nc.vector.tensor_add(numden[:, wsl],
                                                 numden[:, wsl], ps[:])

            # M phase: agg -> MLP(+folded LN) -> residual, per quarter shard
            for q in range(4):
                slq = slice(q * NQ, (q + 1) * NQ)
                dn_lo = t64.tile([H, NQ], dt.bfloat16, tag="t64")
                nc.vector.tensor_scalar_max(dn_lo[:], numden[H:HH, slq], 0.5)
                s_t = t64.tile([H, NQ], dt.bfloat16, tag="t64")
                nc.scalar.activation(s_t[:], dn_lo[:],
                                     AF.Abs_reciprocal_sqrt,
                                     bias=pcol("tiny", H))
                rden = t64.tile([H, NQ], dt.bfloat16, tag="t64")
                nc.vector.tensor_mul(rden[:], s_t[:], s_t[:])
                t1 = t64.tile([H, NQ], dt.bfloat16, tag="t64")
                nc.vector.tensor_mul(t1[:], numden[0:H, slq], rden[:])
                nc.vector.tensor_add(rbf65[0:H, :], t1[:], hT[0:H, slq])
                ycs = t128.tile([HH, NQ], dt.bfloat16, tag="t128")
                rstd = t128.tile([HH, NQ], dt.bfloat16, tag="t128")
                for c in range(CPQ):
                    lo = c * MCH
                    ps1 = mpa.tile([HH, MCH], dt.float32, tag="mma")
                    nc.tensor.matmul(ps1[:], sb_w1[:, l, :],
                                     rbf65[:, lo:lo + MCH])
                    nc.vector.tensor_copy(ycs[:, lo:lo + MCH], ps1[:])
                sq = t128.tile([HH, NQ], dt.bfloat16, tag="t128")
                nc.vector.tensor_mul(sq[:], ycs[:], ycs[:])
                for c in range(CPQ):
                    lo = c * MCH
                    pv = mpa.tile([HH, MCH], dt.float32, tag="mma")
                    nc.tensor.matmul(pv[:], sb_o128[:], sq[:, lo:lo + MCH])
                    nc.scalar.activation(rstd[:, lo:lo + MCH], pv[:],
                                         AF.Abs_reciprocal_sqrt,
                                         bias=pcol("eps"))
                t_t = t128.tile([HH, NQ], dt.bfloat16, tag="t128")
                nc.vector.tensor_mul(t_t[:], ycs[:], rstd[:])
                h1 = t128.tile([HH, NQ], dt.bfloat16, tag="t128")
                nc.scalar.activation(h1[:], t_t[:], AF.Relu,
                                     bias=pcol(("be1", l)),
                                     scale=pcol(("g1", l)))
                if is_first:
                    for c in range(CPQ):
                        lo = c * MCH
                        sl = slice(q * NQ + lo, q * NQ + lo + MCH)
                        ps2 = mpb.tile([H, MCH], dt.float32, tag="mmb")
                        nc.tensor.matmul(ps2[:], sb_w2r[:, :], h1[:, lo:lo + MCH])
                        nc.vector.tensor_scalar_add(hT[0:H, sl], ps2[:],
                                                    pcol("b2r", H))
                else:
                    ycs2 = t64.tile([H, NQ], dt.bfloat16, tag="t64")
                    rstd2 = t64.tile([H, NQ], dt.bfloat16, tag="t64")
                    for c in range(CPQ):
                        lo = c * MCH
                        ps2 = mpb.tile([H, MCH], dt.float32, tag="mmb")
                        nc.tensor.matmul(ps2[:], sb_w2[:, l, :], h1[:, lo:lo + MCH])
                        nc.vector.tensor_scalar_add(ycs2[:, lo:lo + MCH], ps2[:],
                                                    pcol(("b2c", l), H))
                    sq2 = t64.tile([H, NQ], dt.bfloat16, tag="t64")
                    nc.vector.tensor_mul(sq2[:], ycs2[:], ycs2[:])
                    for c in range(CPQ):
                        lo = c * MCH
                        pv2 = mpb.tile([H, MCH], dt.float32, tag="mmb")
                        nc.tensor.matmul(pv2[:], sb_o64[:], sq2[:, lo:lo + MCH])
                        nc.scalar.activation(rstd2[:, lo:lo + MCH], pv2[:],
                                             AF.Abs_reciprocal_sqrt,
                                             bias=pcol("eps", H))
                    t2 = t64.tile([H, NQ], dt.bfloat16, tag="t64")
                    nc.vector.tensor_mul(t2[:], ycs2[:], rstd2[:])
                    c_t = t64.tile([H, NQ], dt.bfloat16, tag="t64")
                    nc.scalar.activation(c_t[:], t2[:], AF.Relu,
                                         bias=pcol(("nb", l), H),
                                         scale=pcol(("ng", l), H))
                    nc.vector.tensor_add(hT[0:H, slq], hT[0:H, slq], c_t[:])

        # ---------------- final head (reuses conv pools) ----------------
        for q in range(4):
            slq = slice(q * NQ, (q + 1) * NQ)
            yc = t128.tile([H, NQ], dt.bfloat16, tag="t128")
            rstd = t64.tile([H, NQ], dt.bfloat16, tag="t64")
            for c in range(CPQ):
                lo = c * MCH
                sl = slice(q * NQ + lo, q * NQ + lo + MCH)
                pmu = mpb.tile([H, MCH], dt.float32, tag="mmb")
                nc.tensor.matmul(pmu[:], sb_o64[:], hT[0:H, sl])
                nc.vector.tensor_sub(yc[:, lo:lo + MCH], hT[0:H, sl], pmu[:])
            sq = t64.tile([H, NQ], dt.bfloat16, tag="t64")
            nc.vector.tensor_mul(sq[:], yc[:], yc[:])
            for c in range(CPQ):
                lo = c * MCH
                pv = mpb.tile([H, MCH], dt.float32, tag="mmb")
                nc.tensor.matmul(pv[:], sb_o64[:], sq[:, lo:lo + MCH])
                nc.scalar.activation(rstd[:, lo:lo + MCH], pv[:],
                                     AF.Abs_reciprocal_sqrt, bias=pcol("eps", H))
            t_t = t64.tile([H, NQ], dt.bfloat16, tag="t64")
            nc.vector.tensor_mul(t_t[:], yc[:], rstd[:])
            f65 = t128.tile([H + 1, NQ], dt.bfloat16, tag="t128")
            nc.vector.memset(f65[H:H + 1, :], 1.0)
            nc.scalar.activation(f65[0:H, :], t_t[:], AF.Relu,
                                 bias=pcol(("nb", 0), H),
                                 scale=pcol(("ng", 0), H))
            for c in range(CPQ):
                lo = c * MCH
                sl = slice(q * NQ + lo, q * NQ + lo + MCH)
                pso = mpa.tile([C, MCH], dt.float32, tag="mma")
                nc.tensor.matmul(pso[:], sb_linw[:, :], f65[:, lo:lo + MCH])
                ot = stp.tile([C, MCH], dt.float32, tag="ot")
                nc.vector.tensor_copy(ot[:], pso[:])
                nc.sync.dma_start(outp_d[:, sl], ot[:])

    nc.compile()
    return nc, NPCOL, PCOL


# --------------------------------------------------------------------------
# host: input packing
# --------------------------------------------------------------------------

def pack_inputs(inputs, structs, NPCOL, PCOL, cfg=CFG):
    bf16 = ml_dtypes.bfloat16
    NSH, NPAD, FPAD = cfg.NSH, cfg.NPAD, cfg.FPAD
    H, HH, C, L = cfg.H, cfg.HH, cfg.C, cfg.L

    x = np.asarray(inputs["x"], np.float32)
    xp = np.zeros((NPAD, FPAD), np.float32)
    xp[: x.shape[0], : x.shape[1]] = x

    encw = np.zeros((FPAD, H), np.float32)
    encw[: cfg.F_IN] = np.asarray(inputs["enc_W"], np.float32)
    encw = np.ascontiguousarray(
        encw.reshape(FPAD // 128, 128, H).transpose(1, 0, 2)).astype(bf16)

    # W1 with LN mean-centering folded in + centered bias as a 65th row
    W1 = np.asarray(inputs["W1"], np.float32)          # [L, H, HH]
    b1 = np.asarray(inputs["b1"], np.float32)          # [L, HH]
    W1c = W1 - W1.mean(axis=2, keepdims=True)
    b1c = b1 - b1.mean(axis=1, keepdims=True)
    w1s = np.concatenate([W1c, b1c[:, None, :]], axis=1)   # [L, H+1, HH]
    w1s = np.ascontiguousarray(w1s.transpose(1, 0, 2)).astype(bf16)

    W2 = np.asarray(inputs["W2"], np.float32)          # [L, HH, H]
    W2c = W2 - W2.mean(axis=2, keepdims=True)
    w2s = np.ascontiguousarray(W2c.transpose(1, 0, 2)).astype(bf16)
    w2r = np.ascontiguousarray(W2[0]).astype(bf16)     # uncentered, conv 0

    linw = np.asarray(inputs["lin_W"], np.float32)
    linb = np.asarray(inputs["lin_b"], np.float32)
    linw65 = np.concatenate([linw, linb[None, :]], axis=0).astype(bf16)
    ident = np.eye(128, dtype=bf16)

    b2 = np.asarray(inputs["b2"], np.float32)          # [L, H]
    b2c = b2 - b2.mean(axis=1, keepdims=True)

    params = np.zeros((128, NPCOL), np.float32)
    params[:H, PCOL["encb"]] = inputs["enc_b"]
    params[:H, PCOL["b2r"]] = b2[0]
    params[:, PCOL["eps"]] = LN_EPS
    params[:, PCOL["tiny"]] = 1e-30
    for l in range(L):
        params[:, PCOL[("g1", l)]] = inputs["g1"][l]
        params[:, PCOL[("be1", l)]] = inputs["be1"][l]
        params[:H, PCOL[("b2c", l)]] = b2c[l]
        params[:H, PCOL[("ng", l)]] = inputs["ng"][l]
        params[:H, PCOL[("nb", l)]] = inputs["nb"][l]
        params[:, PCOL[("t", l)]] = float(np.asarray(inputs["t"][l]))

    in_maps = []
    for k in range(NCORES):
        xs = np.ascontiguousarray(
            xp[k * NSH:(k + 1) * NSH].T).astype(bf16)
        gi = structs["gidx"][k]
        gw = np.tile(np.ascontiguousarray(gi.reshape(-1, 16).T), (8, 1))
        in_maps.append({
            "xt": xs, "encw": encw, "w1": w1s, "w2": w2s, "w2r": w2r,
            "linw": linw65, "ident": ident, "params": params, "gidx": gw,
            "rmat": structs["rmat"][k],
        })
    return in_maps


def _run(inputs, cfg=CFG, trace=False, tmpdir=None):
    import sys
    sys.path.insert(0, "/root/problem")
    from concourse.bass_utils import run_bass_kernel_spmd

    structs = build_edge_structs(inputs["edge_index"], cfg)
    nc, NPCOL, PCOL = build_nc(structs, cfg)
    in_maps = pack_inputs(inputs, structs, NPCOL, PCOL, cfg)
    res = run_bass_kernel_spmd(nc, in_maps, list(range(NCORES)), trace=trace,
                               tmpdir=tmpdir)
    outs = [res.results[k]["outp"] for k in range(NCORES)]  # [C, NSH] each
    full = np.concatenate(outs, axis=1).T                   # [NPAD, C]
    return np.ascontiguousarray(full[: cfg.N]).astype(np.float32), res


def kernel(**inputs) -> np.ndarray:
    out, _ = _run(inputs)
    return out


# revision 19
# speedup vs baseline: 1.9250x; 1.1468x over previous
"""DeeperGCN Trainium2 kernel (8 NeuronCores, SPMD).

Strategy:
  - Nodes padded to NPAD = 8*NSH and sharded by dst across 8 cores.
  - Per conv layer, per-node messages u = m*exp(t*m), w = exp(t*m)
    (m = relu(h)+eps) are computed shard-wise (feature-major), transposed to
    node-major and AllGathered in FOUR row-quarters (double-buffered across
    layers) so edge gathers can start as soon as their quarter's table lands.
  - Edge aggregation runs as 4 passes (one per src quarter): dma_gather of
    uv rows by src (int16 indices, one SWDGE queue per pass), then a
    segmented sum over dst via TensorE matmuls with one-hot fp8 R matrices
    (host-precomputed, streamed); per-pass partial sums accumulate into a
    bf16 numden master (copy on pass 0, DVE add after).
  - MLP+LN evaluated feature-major with LN mean-centering and biases FOLDED
    into the weights host-side (ones-row trick for b1c), variance via
    ones-matmul of the square, rstd via ACT Abs_reciprocal_sqrt, all
    elementwise in bf16 at quarter-shard width.
Host preprocessing (edge bucketing, R matrices, transpose/cast of x,
weight centering) is numpy; only device NEFF time counts.
"""
import numpy as np
import ml_dtypes
from dataclasses import dataclass

EPS_MSG = 1e-7
LN_EPS = 1e-5
NCORES = 8


@dataclass
class Cfg:
    N: int = 100000
    E: int = 1000000
    F_IN: int = 500
    H: int = 64
    C: int = 3
    L: int = 3
    NSH: int = 12544            # nodes/core, multiple of 128
    NSW: int = 4                # src quarters (one AllGather + GS pass each)
    SLOTS_G: int = 56           # gather-buffer chunk slots per group
    MCH: int = 448              # MLP node-chunk (<=512)

    @property
    def NPAD(self):
        return NCORES * self.NSH

    @property
    def NW(self):
        return self.NSH // 128

    @property
    def WSZ(self):
        w = self.NPAD // self.NSW
        assert w <= 32768
        return w

    @property
    def HH(self):
        return 2 * self.H

    @property
    def FPAD(self):
        return ((self.F_IN + 127) // 128) * 128

    @property
    def NQ(self):
        return self.NSH // 4        # M/U-phase quarter width (3136)


CFG = Cfg()


def _cdiv(a, b):
    return (a + b - 1) // b


# --------------------------------------------------------------------------
# host: edge structures
# --------------------------------------------------------------------------

def build_edge_structs(edge_index, cfg=CFG):
    src = np.asarray(edge_index[0], np.int64)
    dst = np.asarray(edge_index[1], np.int64)
    NSH, NW, NSW, WSZ = cfg.NSH, cfg.NW, cfg.NSW, cfg.WSZ
    core = dst // NSH
    dstloc = dst % NSH
    w = dstloc // 128
    dcol = dstloc % 128
    s = src // WSZ
    srcloc = src % WSZ
    assert srcloc.max() < 32768

    counts = np.zeros((NCORES, NW, NSW), np.int64)
    np.add.at(counts, (core, w, s), 1)
    PBc = _cdiv(counts, 128).max(axis=0)           # [NW, NSW]
    PBc = np.maximum(PBc, 1)

    # window groups with <= SLOTS_G total chunks (across all 4 src windows)
    win_chunks = PBc.sum(axis=1)
    groups, w0, acc = [], 0, 0
    for wi in range(NW):
        c = int(win_chunks[wi])
        if acc and acc + c > cfg.SLOTS_G:
            groups.append((w0, wi))
            w0, acc = wi, 0
        acc += c
    groups.append((w0, NW))

    IW = [int(PBc[:, si].sum()) * 128 for si in range(NSW)]
    CTOT = int(PBc.sum())
    IW_base = np.concatenate([[0], np.cumsum(IW)]).astype(np.int64)

    gx_off = np.zeros((NW, NSW), np.int64)
    for si in range(NSW):
        o = 0
        for wi in range(NW):
            gx_off[wi, si] = o
            o += int(PBc[wi, si]) * 128
        assert o == IW[si]
    # R chunk id == global gather slot index
    ch_off = (IW_base[:-1][None, :] + gx_off) // 128   # [NW, NSW]

    order = np.lexsort((w, s, core))
    src_s, core_s, s_s, w_s, dcol_s = (
        srcloc[order], core[order], s[order], w[order], dcol[order])

    gidx_all, rmat_all = [], []
    for k in range(NCORES):
        sel = core_s == k
        ks, kw, kdc, ksrc = s_s[sel], w_s[sel], dcol_s[sel], src_s[sel]
        gidx = [np.zeros(IW[si], np.int16) for si in range(NSW)]
        rmat = np.zeros((128, CTOT, 128), ml_dtypes.float8_e4m3)
        for si in range(NSW):
            insel = ks == si
            kwsi, kdcsi, ksrcsi = kw[insel], kdc[insel], ksrc[insel]
            bnd = np.searchsorted(kwsi, np.arange(NW + 1))
            for wi in range(NW):
                a, b = bnd[wi], bnd[wi + 1]
                n = b - a
                cap = int(PBc[wi, si]) * 128
                assert n <= cap
                o = gx_off[wi, si]
                gidx[si][o: o + n] = ksrcsi[a:b].astype(np.int16)
                j = np.arange(n)
                rmat[j % 128, ch_off[wi, si] + j // 128, kdcsi[a:b]] = 1.0
        gidx_all.append(np.concatenate(gidx))
        rmat_all.append(rmat)

    return dict(PBc=PBc, groups=groups, IW=IW, CTOT=CTOT,
                gx_off=gx_off, IW_base=IW_base,
                gidx=gidx_all, rmat=rmat_all)


# --------------------------------------------------------------------------
# device builder
# --------------------------------------------------------------------------

def build_nc(structs, cfg=CFG):
    import concourse.bass as bass
    import concourse.tile as tile
    from concourse import bacc, mybir
    from contextlib import ExitStack

    dt = mybir.dt
    AF = mybir.ActivationFunctionType
    AL = mybir.AluOpType
    PBc, groups, IW, CTOT = (structs["PBc"], structs["groups"],
                             structs["IW"], structs["CTOT"])
    gx_off, IW_base = structs["gx_off"], structs["IW_base"]
    NSH, NW, NSW, WSZ = cfg.NSH, cfg.NW, cfg.NSW, cfg.WSZ
    H, HH, C, L, FPAD = cfg.H, cfg.HH, cfg.C, cfg.L, cfg.FPAD
    MCH, NQ = cfg.MCH, cfg.NQ
    NMC = NSH // MCH
    assert NMC * MCH == NSH
    CPQ = NQ // MCH
    assert CPQ * MCH == NQ
    IWALL = sum(IW)

    PCOL = {"encb": 0, "b2r": 1, "eps": 2, "tiny": 3, "zero": 4}
    nc_col = 5
    for l in range(L):
        for nm in ("g1", "be1", "b2c", "ng", "nb", "t"):
            PCOL[(nm, l)] = nc_col
            nc_col += 1
    NPCOL = nc_col

    nc = bacc.Bacc("TRN2", num_swdge_queues=4, dynamic_dma_scratch_size=32768)
    xt_d = nc.declare_dram_parameter("xt", [FPAD, NSH], dt.bfloat16, isOutput=False)
    encw_d = nc.declare_dram_parameter("encw", [128, FPAD // 128, H], dt.bfloat16, isOutput=False)
    w1_d = nc.declare_dram_parameter("w1", [H + 1, L, HH], dt.bfloat16, isOutput=False)
    w2_d = nc.declare_dram_parameter("w2", [HH, L, H], dt.bfloat16, isOutput=False)
    w2r_d = nc.declare_dram_parameter("w2r", [HH, H], dt.bfloat16, isOutput=False)
    linw_d = nc.declare_dram_parameter("linw", [H + 1, C], dt.bfloat16, isOutput=False)
    ident_d = nc.declare_dram_parameter("ident", [128, 128], dt.bfloat16, isOutput=False)
    params_d = nc.declare_dram_parameter("params", [128, NPCOL], dt.float32, isOutput=False)
    gidx_d = nc.declare_dram_parameter("gidx", [128, IWALL // 16], dt.int16, isOutput=False)
    rmat_d = nc.declare_dram_parameter("rmat", [128, CTOT, 128], dt.float8e4, isOutput=False)
    outp_d = nc.declare_dram_parameter("outp", [C, NSH], dt.float32, isOutput=True)

    uvshard = nc.dram_tensor("uvshard", [NSH, HH], dt.bfloat16)
    tabs = [nc.dram_tensor(f"uvtab{i}", [cfg.NPAD, HH], dt.bfloat16,
                           addr_space="Shared") for i in range(2)]

    with tile.TileContext(nc) as tc, ExitStack() as ctx:
        const = ctx.enter_context(tc.tile_pool(name="const", bufs=1))
        sb_par = const.tile([128, NPCOL], dt.float32)
        nc.sync.dma_start(sb_par[:], params_d[:])
        sb_encw = const.tile([128, FPAD // 128, H], dt.bfloat16)
        nc.sync.dma_start(sb_encw[:], encw_d[:])
        sb_w1 = const.tile([H + 1, L, HH], dt.bfloat16)
        nc.sync.dma_start(sb_w1[:], w1_d[:])
        sb_w2 = const.tile([HH, L, H], dt.bfloat16)
        nc.sync.dma_start(sb_w2[:], w2_d[:])
        sb_w2r = const.tile([HH, H], dt.bfloat16)
        nc.sync.dma_start(sb_w2r[:], w2r_d[:])
        sb_linw = const.tile([H + 1, C], dt.bfloat16)
        nc.sync.dma_start(sb_linw[:], linw_d[:])
        sb_id = const.tile([128, 128], dt.bfloat16)
        nc.sync.dma_start(sb_id[:], ident_d[:])
        sb_o128 = const.tile([128, 128], dt.bfloat16)
        nc.vector.memset(sb_o128[:], 1.0 / 128)
        sb_o64 = const.tile([H, H], dt.bfloat16)
        nc.vector.memset(sb_o64[:], 1.0 / H)
        sb_gidx = const.tile([128, IWALL // 16], dt.int16)
        nc.sync.dma_start(sb_gidx[:], gidx_d[:])

        def pcol(key, rows=128):
            cidx = PCOL[key]
            return sb_par[0:rows, cidx: cidx + 1]

        master = ctx.enter_context(tc.tile_pool(name="master", bufs=1))
        hT = master.tile([H + 1, NSH], dt.bfloat16)     # row H == 1.0
        numden = master.tile([HH, NSH], dt.bfloat16)
        uv2 = master.tile([128, NSH // 2], dt.bfloat16)
        rbf65 = master.tile([H + 1, NQ], dt.bfloat16)   # row H == 1.0
        nc.vector.memset(hT[H:H + 1, :], 1.0)
        nc.vector.memset(rbf65[H:H + 1, :], 1.0)

        # ---------------- encoder: hT = (x @ enc_W + b)^T ----------------
        with tc.tile_pool(name="enc", bufs=3) as ep, \
             tc.tile_pool(name="encps", bufs=2, space="PSUM") as pp:
            for c in range(NMC):
                sl = slice(c * MCH, (c + 1) * MCH)
                xtile = ep.tile([128, FPAD // 128, MCH], dt.bfloat16)
                for fc in range(FPAD // 128):
                    nc.sync.dma_start(
                        xtile[:, fc, :], xt_d[fc * 128:(fc + 1) * 128, sl])
                ps = pp.tile([H, MCH], dt.float32)
                for fc in range(FPAD // 128):
                    nc.tensor.matmul(ps[:], sb_encw[:, fc, :], xtile[:, fc, :],
                                     start=(fc == 0), stop=(fc == FPAD // 128 - 1))
                nc.vector.tensor_scalar_add(hT[0:H, sl], ps[:], pcol("encb", H))

        # persistent pools for the conv pipeline
        t64 = ctx.enter_context(tc.tile_pool(name="t64", bufs=4))
        t128 = ctx.enter_context(tc.tile_pool(name="t128", bufs=4))
        utp = ctx.enter_context(tc.tile_pool(name="utp", bufs=1))
        stp = ctx.enter_context(tc.tile_pool(name="stp", bufs=2))
        gp = ctx.enter_context(tc.tile_pool(name="gp", bufs=2))
        rp = ctx.enter_context(tc.tile_pool(name="rp", bufs=2))
        tpp = ctx.enter_context(tc.tile_pool(name="tpp", bufs=2, space="PSUM"))
        gsp = ctx.enter_context(tc.tile_pool(name="gsp", bufs=2, space="PSUM"))
        mpa = ctx.enter_context(tc.tile_pool(name="mpa", bufs=2, space="PSUM"))
        mpb = ctx.enter_context(tc.tile_pool(name="mpb", bufs=2, space="PSUM"))

        # ---------------- conv layers ----------------
        conv_params = [0] + list(range(L))          # [0, 0, 1, 2]
        for conv, l in enumerate(conv_params):
            is_first = conv == 0
            tab = tabs[conv % 2]

            # U phase: uv2 rows 0:H = u = m*exp(t*m), rows H:HH = w
            # (half-shard staging buffer; transpose each half then reuse)
            NWH = NW // 2
            for half in range(2):
                for qq in range(2):
                    q = half * 2 + qq
                    slq = slice(q * NQ, (q + 1) * NQ)
                    sll = slice(qq * NQ, (qq + 1) * NQ)
                    m_t = utp.tile([H, NQ], dt.bfloat16, tag="mt")
                    nc.vector.tensor_scalar(
                        m_t[:], hT[0:H, slq], 0.0, EPS_MSG,
                        AL.max, AL.add)
                    wq = utp.tile([H, NQ], dt.bfloat16, tag="wq")
                    nc.scalar.activation(wq[:], m_t[:], AF.Exp,
                                         scale=pcol(("t", l), H))
                    # partition-crossing copy (tensor_scalar base shift)
                    nc.vector.tensor_scalar_add(uv2[H:HH, sll], wq[:],
                                                pcol("zero", H))
                    nc.vector.tensor_mul(uv2[0:H, sll], m_t[:], wq[:])
                for ntl in range(NWH):
                    nt = half * NWH + ntl
                    tp = tpp.tile([128, HH], dt.bfloat16, tag="tp")
                    nc.tensor.transpose(
                        tp[:], uv2[:, ntl * 128:(ntl + 1) * 128], sb_id[:])
                    st = stp.tile([128, HH], dt.bfloat16, tag="st")
                    if nt % 2 == 0:
                        nc.scalar.copy(st[:], tp[:])
                    else:
                        nc.vector.tensor_copy(st[:], tp[:])
                    nc.sync.dma_start(uvshard[nt * 128:(nt + 1) * 128, :],
                                      st[:])
            nc.gpsimd.collective_compute(
                "AllGather", mybir.AluOpType.bypass,
                replica_groups=[list(range(NCORES))],
                ins=[uvshard[:, :]], outs=[tab[:, :]])

            # G+S: per window-group, 4 src-window sub-gathers (one per
            # SWDGE queue) + fp8 R matmuls accumulating all of a window's
            # chunks in PSUM, then one copy into numden
            for (w0, w1) in groups:
                slots_s = [int(PBc[w0:w1, si].sum()) for si in range(NSW)]
                tot = sum(slots_s)
                gbuf = gp.tile([128, tot, HH], dt.bfloat16, tag="gbuf")
                rtile = rp.tile([128, tot, 128], dt.float8e4, tag="rt")
                off = 0
                reg_off = []
                for si in range(NSW):
                    reg_off.append(off)
                    nsl = slots_s[si]
                    if nsl:
                        a = int(IW_base[si] + gx_off[w0, si])
                        nidx = nsl * 128
                        nc.gpsimd.dma_gather(
                            gbuf[:, off: off + nsl, :],
                            tab[si * WSZ: (si + 1) * WSZ, :],
                            sb_gidx[:, a // 16: (a + nidx) // 16],
                            nidx, nidx, HH, single_packet=False,
                            queue_num=si)
                        nc.sync.dma_start(
                            rtile[:, off: off + nsl, :],
                            rmat_d[:, a // 128: a // 128 + nsl, :])
                    off += nsl
                for wi in range(w0, w1):
                    nchw = int(PBc[wi].sum())
                    ps = gsp.tile([HH, 128], dt.float32, tag="gs")
                    done = 0
                    for si in range(NSW):
                        base = (reg_off[si]
                                + int(gx_off[wi, si] - gx_off[w0, si]) // 128)
                        for j in range(int(PBc[wi, si])):
                            nc.tensor.matmul(
                                ps[:], gbuf[:, base + j, :],
                                rtile[:, base + j, :],
                                start=(done == 0), stop=(done == nchw - 1))
                            done += 1
                    wsl = slice(wi * 128, (wi + 1) * 128)
                    if wi % 2 == 0:
                        nc.scalar.copy(numden[:, wsl], ps[:])
                    else:
                        nc.vector.tensor_copy(numden[:, wsl], ps[:])

            # M phase: agg -> MLP(+folded LN) -> residual, per quarter shard
            for q in range(4):
                slq = slice(q * NQ, (q + 1) * NQ)
                dn_lo = t64.tile([H, NQ], dt.bfloat16, tag="t64")
                nc.vector.tensor_scalar_max(dn_lo[:], numden[H:HH, slq], 0.5)
                s_t = t64.tile([H, NQ], dt.bfloat16, tag="t64")
                nc.scalar.activation(s_t[:], dn_lo[:],
                                     AF.Abs_reciprocal_sqrt,
                                     bias=pcol("tiny", H))
                rden = t64.tile([H, NQ], dt.bfloat16, tag="t64")
                nc.vector.tensor_mul(rden[:], s_t[:], s_t[:])
                t1 = t64.tile([H, NQ], dt.bfloat16, tag="t64")
                nc.vector.tensor_mul(t1[:], numden[0:H, slq], rden[:])
                nc.vector.tensor_add(rbf65[0:H, :], t1[:], hT[0:H, slq])
                ycs = t128.tile([HH, NQ], dt.bfloat16, tag="t128")
                rstd = t128.tile([HH, NQ], dt.bfloat16, tag="t128")
                for c in range(CPQ):
                    lo = c * MCH
                    ps1 = mpa.tile([HH, MCH], dt.float32, tag="mma")
                    nc.tensor.matmul(ps1[:], sb_w1[:, l, :],
                                     rbf65[:, lo:lo + MCH])
                    nc.vector.tensor_copy(ycs[:, lo:lo + MCH], ps1[:])
                sq = t128.tile([HH, NQ], dt.bfloat16, tag="t128")
                nc.vector.tensor_mul(sq[:], ycs[:], ycs[:])
                for c in range(CPQ):
                    lo = c * MCH
                    pv = mpa.tile([HH, MCH], dt.float32, tag="mma")
                    nc.tensor.matmul(pv[:], sb_o128[:], sq[:, lo:lo + MCH])
                    nc.scalar.activation(rstd[:, lo:lo + MCH], pv[:],
                                         AF.Abs_reciprocal_sqrt,
                                         bias=pcol("eps"))
                t_t = t128.tile([HH, NQ], dt.bfloat16, tag="t128")
                nc.vector.tensor_mul(t_t[:], ycs[:], rstd[:])
                h1 = t128.tile([HH, NQ], dt.bfloat16, tag="t128")
                nc.scalar.activation(h1[:], t_t[:], AF.Relu,
                                     bias=pcol(("be1", l)),
                                     scale=pcol(("g1", l)))
                if is_first:
                    for c in range(CPQ):
                        lo = c * MCH
                        sl = slice(q * NQ + lo, q * NQ + lo + MCH)
                        ps2 = mpb.tile([H, MCH], dt.float32, tag="mmb")
                        nc.tensor.matmul(ps2[:], sb_w2r[:, :], h1[:, lo:lo + MCH])
                        nc.vector.tensor_scalar_add(hT[0:H, sl], ps2[:],
                                                    pcol("b2r", H))
                else:
                    ycs2 = t64.tile([H, NQ], dt.bfloat16, tag="t64")
                    rstd2 = t64.tile([H, NQ], dt.bfloat16, tag="t64")
                    for c in range(CPQ):
                        lo = c * MCH
                        ps2 = mpb.tile([H, MCH], dt.float32, tag="mmb")
                        nc.tensor.matmul(ps2[:], sb_w2[:, l, :], h1[:, lo:lo + MCH])
                        nc.vector.tensor_scalar_add(ycs2[:, lo:lo + MCH], ps2[:],
                                                    pcol(("b2c", l), H))
                    sq2 = t64.tile([H, NQ], dt.bfloat16, tag="t64")
                    nc.vector.tensor_mul(sq2[:], ycs2[:], ycs2[:])
                    for c in range(CPQ):
                        lo = c * MCH
                        pv2 = mpb.tile([H, MCH], dt.float32, tag="mmb")
                        nc.tensor.matmul(pv2[:], sb_o64[:], sq2[:, lo:lo + MCH])
                        nc.scalar.activation(rstd2[:, lo:lo + MCH], pv2[:],
                                             AF.Abs_reciprocal_sqrt,
                                             bias=pcol("eps", H))
                    t2 = t64.tile([H, NQ], dt.bfloat16, tag="t64")
                    nc.vector.tensor_mul(t2[:], ycs2[:], rstd2[:])
                    c_t = t64.tile([H, NQ], dt.bfloat16, tag="t64")
                    nc.scalar.activation(c_t[:], t2[:], AF.Relu,
                                         bias=pcol(("nb", l), H),
                                         scale=pcol(("ng", l), H))
                    nc.vector.tensor_add(hT[0:H, slq], hT[0:H, slq], c_t[:])

        # ---------------- final head (reuses conv pools) ----------------
        for q in range(4):
            slq = slice(q * NQ, (q + 1) * NQ)
            yc = t128.tile([H, NQ], dt.bfloat16, tag="t128")
            rstd = t64.tile([H, NQ], dt.bfloat16, tag="t64")
            for c in range(CPQ):
                lo = c * MCH
                sl = slice(q * NQ + lo, q * NQ + lo + MCH)
                pmu = mpb.tile([H, MCH], dt.float32, tag="mmb")
                nc.tensor.matmul(pmu[:], sb_o64[:], hT[0:H, sl])
                nc.vector.tensor_sub(yc[:, lo:lo + MCH], hT[0:H, sl], pmu[:])
            sq = t64.tile([H, NQ], dt.bfloat16, tag="t64")
            nc.vector.tensor_mul(sq[:], yc[:], yc[:])
            for c in range(CPQ):
                lo = c * MCH
                pv = mpb.tile([H, MCH], dt.float32, tag="mmb")
                nc.tensor.matmul(pv[:], sb_o64[:], sq[:, lo:lo + MCH])
                nc.scalar.activation(rstd[:, lo:lo + MCH], pv[:],
                                     AF.Abs_reciprocal_sqrt, bias=pcol("eps", H))
            t_t = t64.tile([H, NQ], dt.bfloat16, tag="t64")
            nc.vector.tensor_mul(t_t[:], yc[:], rstd[:])
            f65 = t128.tile([H + 1, NQ], dt.bfloat16, tag="t128")
            nc.vector.memset(f65[H:H + 1, :], 1.0)
            nc.scalar.activation(f65[0:H, :], t_t[:], AF.Relu,
                                 bias=pcol(("nb", 0), H),
                                 scale=pcol(("ng", 0), H))
            for c in range(CPQ):
                lo = c * MCH
                sl = slice(q * NQ + lo, q * NQ + lo + MCH)
                pso = mpa.tile([C, MCH], dt.float32, tag="mma")
                nc.tensor.matmul(pso[:], sb_linw[:, :], f65[:, lo:lo + MCH])
                ot = stp.tile([C, MCH], dt.float32, tag="ot")
                nc.vector.tensor_copy(ot[:], pso[:])
                nc.sync.dma_start(outp_d[:, sl], ot[:])

    nc.compile()
    return nc, NPCOL, PCOL


# --------------------------------------------------------------------------
# host: input packing
# --------------------------------------------------------------------------

def pack_inputs(inputs, structs, NPCOL, PCOL, cfg=CFG):
    bf16 = ml_dtypes.bfloat16
    NSH, NPAD, FPAD = cfg.NSH, cfg.NPAD, cfg.FPAD
    H, HH, C, L = cfg.H, cfg.HH, cfg.C, cfg.L

    x = np.asarray(inputs["x"], np.float32)
    xp = np.zeros((NPAD, FPAD), np.float32)
    xp[: x.shape[0], : x.shape[1]] = x

    encw = np.zeros((FPAD, H), np.float32)
    encw[: cfg.F_IN] = np.asarray(inputs["enc_W"], np.float32)
    encw = np.ascontiguousarray(
        encw.reshape(FPAD // 128, 128, H).transpose(1, 0, 2)).astype(bf16)

    # W1 with LN mean-centering folded in + centered bias as a 65th row
    W1 = np.asarray(inputs["W1"], np.float32)          # [L, H, HH]
    b1 = np.asarray(inputs["b1"], np.float32)          # [L, HH]
    W1c = W1 - W1.mean(axis=2, keepdims=True)
    b1c = b1 - b1.mean(axis=1, keepdims=True)
    w1s = np.concatenate([W1c, b1c[:, None, :]], axis=1)   # [L, H+1, HH]
    w1s = np.ascontiguousarray(w1s.transpose(1, 0, 2)).astype(bf16)

    W2 = np.asarray(inputs["W2"], np.float32)          # [L, HH, H]
    W2c = W2 - W2.mean(axis=2, keepdims=True)
    w2s = np.ascontiguousarray(W2c.transpose(1, 0, 2)).astype(bf16)
    w2r = np.ascontiguousarray(W2[0]).astype(bf16)     # uncentered, conv 0

    linw = np.asarray(inputs["lin_W"], np.float32)
    linb = np.asarray(inputs["lin_b"], np.float32)
    linw65 = np.concatenate([linw, linb[None, :]], axis=0).astype(bf16)
    ident = np.eye(128, dtype=bf16)

    b2 = np.asarray(inputs["b2"], np.float32)          # [L, H]
    b2c = b2 - b2.mean(axis=1, keepdims=True)

    params = np.zeros((128, NPCOL), np.float32)
    params[:H, PCOL["encb"]] = inputs["enc_b"]
    params[:H, PCOL["b2r"]] = b2[0]
    params[:, PCOL["eps"]] = LN_EPS
    params[:, PCOL["tiny"]] = 1e-30
    for l in range(L):
        params[:, PCOL[("g1", l)]] = inputs["g1"][l]
        params[:, PCOL[("be1", l)]] = inputs["be1"][l]
        params[:H, PCOL[("b2c", l)]] = b2c[l]
        params[:H, PCOL[("ng", l)]] = inputs["ng"][l]
        params[:H, PCOL[("nb", l)]] = inputs["nb"][l]
        params[:, PCOL[("t", l)]] = float(np.asarray(inputs["t"][l]))

    in_maps = []
    for k in range(NCORES):
        xs = np.ascontiguousarray(
            xp[k * NSH:(k + 1) * NSH].T).astype(bf16)
        gi = structs["gidx"][k]
        gw = np.tile(np.ascontiguousarray(gi.reshape(-1, 16).T), (8, 1))
        in_maps.append({
            "xt": xs, "encw": encw, "w1": w1s, "w2": w2s, "w2r": w2r,
            "linw": linw65, "ident": ident, "params": params, "gidx": gw,
            "rmat": structs["rmat"][k],
        })
    return in_maps


def _run(inputs, cfg=CFG, trace=False, tmpdir=None):
    import sys
    sys.path.insert(0, "/root/problem")
    from concourse.bass_utils import run_bass_kernel_spmd

    structs = build_edge_structs(inputs["edge_index"], cfg)
    nc, NPCOL, PCOL = build_nc(structs, cfg)
    in_maps = pack_inputs(inputs, structs, NPCOL, PCOL, cfg)
    res = run_bass_kernel_spmd(nc, in_maps, list(range(NCORES)), trace=trace,
                               tmpdir=tmpdir)
    outs = [res.results[k]["outp"] for k in range(NCORES)]  # [C, NSH] each
    full = np.concatenate(outs, axis=1).T                   # [NPAD, C]
    return np.ascontiguousarray(full[: cfg.N]).astype(np.float32), res


def kernel(**inputs) -> np.ndarray:
    out, _ = _run(inputs)
    return out


# revision 22
# speedup vs baseline: 1.9288x; 1.0020x over previous
"""DeeperGCN Trainium2 kernel (8 NeuronCores, SPMD).

Strategy:
  - Nodes padded to NPAD = 8*NSH and sharded by dst across 8 cores.
  - Per conv layer, per-node messages u = m*exp(t*m), w = exp(t*m)
    (m = relu(h)+eps) are computed shard-wise (feature-major), transposed to
    node-major and AllGathered in FOUR row-quarters (double-buffered across
    layers) so edge gathers can start as soon as their quarter's table lands.
  - Edge aggregation runs as 4 passes (one per src quarter): dma_gather of
    uv rows by src (int16 indices, one SWDGE queue per pass), then a
    segmented sum over dst via TensorE matmuls with one-hot fp8 R matrices
    (host-precomputed, streamed); per-pass partial sums accumulate into a
    bf16 numden master (copy on pass 0, DVE add after).
  - MLP+LN evaluated feature-major with LN mean-centering and biases FOLDED
    into the weights host-side (ones-row trick for b1c), variance via
    ones-matmul of the square, rstd via ACT Abs_reciprocal_sqrt, all
    elementwise in bf16 at quarter-shard width.
Host preprocessing (edge bucketing, R matrices, transpose/cast of x,
weight centering) is numpy; only device NEFF time counts.
"""
import numpy as np
import ml_dtypes
from dataclasses import dataclass

EPS_MSG = 1e-7
LN_EPS = 1e-5
NCORES = 8


@dataclass
class Cfg:
    N: int = 100000
    E: int = 1000000
    F_IN: int = 500
    H: int = 64
    C: int = 3
    L: int = 3
    NSH: int = 12544            # nodes/core, multiple of 128
    NSW: int = 4                # src quarters (one AllGather + GS pass each)
    SLOTS_G: int = 56           # gather-buffer chunk slots per group
    MCH: int = 448              # MLP node-chunk (<=512)

    @property
    def NPAD(self):
        return NCORES * self.NSH

    @property
    def NW(self):
        return self.NSH // 128

    @property
    def WSZ(self):
        w = self.NPAD // self.NSW
        assert w <= 32768
        return w

    @property
    def HH(self):
        return 2 * self.H

    @property
    def FPAD(self):
        return ((self.F_IN + 127) // 128) * 128

    @property
    def NQ(self):
        return self.NSH // 4        # M/U-phase quarter width (3136)


CFG = Cfg()


def _cdiv(a, b):
    return (a + b - 1) // b


# --------------------------------------------------------------------------
# host: edge structures
# --------------------------------------------------------------------------

def build_edge_structs(edge_index, cfg=CFG):
    src = np.asarray(edge_index[0], np.int64)
    dst = np.asarray(edge_index[1], np.int64)
    NSH, NW, NSW, WSZ = cfg.NSH, cfg.NW, cfg.NSW, cfg.WSZ
    core = dst // NSH
    dstloc = dst % NSH
    w = dstloc // 128
    dcol = dstloc % 128
    s = src // WSZ
    srcloc = src % WSZ
    assert srcloc.max() < 32768

    counts = np.zeros((NCORES, NW, NSW), np.int64)
    np.add.at(counts, (core, w, s), 1)
    PBc = _cdiv(counts, 128).max(axis=0)           # [NW, NSW]
    PBc = np.maximum(PBc, 1)

    # window groups with <= SLOTS_G total chunks (across all 4 src windows)
    win_chunks = PBc.sum(axis=1)
    groups, w0, acc = [], 0, 0
    for wi in range(NW):
        c = int(win_chunks[wi])
        if acc and acc + c > cfg.SLOTS_G:
            groups.append((w0, wi))
            w0, acc = wi, 0
        acc += c
    groups.append((w0, NW))

    IW = [int(PBc[:, si].sum()) * 128 for si in range(NSW)]
    CTOT = int(PBc.sum())
    IW_base = np.concatenate([[0], np.cumsum(IW)]).astype(np.int64)

    gx_off = np.zeros((NW, NSW), np.int64)
    for si in range(NSW):
        o = 0
        for wi in range(NW):
            gx_off[wi, si] = o
            o += int(PBc[wi, si]) * 128
        assert o == IW[si]
    # R chunk id == global gather slot index
    ch_off = (IW_base[:-1][None, :] + gx_off) // 128   # [NW, NSW]

    order = np.lexsort((w, s, core))
    src_s, core_s, s_s, w_s, dcol_s = (
        srcloc[order], core[order], s[order], w[order], dcol[order])

    gidx_all, rmat_all = [], []
    for k in range(NCORES):
        sel = core_s == k
        ks, kw, kdc, ksrc = s_s[sel], w_s[sel], dcol_s[sel], src_s[sel]
        gidx = [np.zeros(IW[si], np.int16) for si in range(NSW)]
        rmat = np.zeros((128, CTOT, 128), ml_dtypes.float8_e4m3)
        for si in range(NSW):
            insel = ks == si
            kwsi, kdcsi, ksrcsi = kw[insel], kdc[insel], ksrc[insel]
            bnd = np.searchsorted(kwsi, np.arange(NW + 1))
            for wi in range(NW):
                a, b = bnd[wi], bnd[wi + 1]
                n = b - a
                cap = int(PBc[wi, si]) * 128
                assert n <= cap
                o = gx_off[wi, si]
                gidx[si][o: o + n] = ksrcsi[a:b].astype(np.int16)
                j = np.arange(n)
                rmat[j % 128, ch_off[wi, si] + j // 128, kdcsi[a:b]] = 1.0
        gidx_all.append(np.concatenate(gidx))
        rmat_all.append(rmat)

    return dict(PBc=PBc, groups=groups, IW=IW, CTOT=CTOT,
                gx_off=gx_off, IW_base=IW_base,
                gidx=gidx_all, rmat=rmat_all)


# --------------------------------------------------------------------------
# device builder
# --------------------------------------------------------------------------

def build_nc(structs, cfg=CFG):
    import concourse.bass as bass
    import concourse.tile as tile
    from concourse import bacc, mybir
    from contextlib import ExitStack

    dt = mybir.dt
    AF = mybir.ActivationFunctionType
    AL = mybir.AluOpType
    PBc, groups, IW, CTOT = (structs["PBc"], structs["groups"],
                             structs["IW"], structs["CTOT"])
    gx_off, IW_base = structs["gx_off"], structs["IW_base"]
    NSH, NW, NSW, WSZ = cfg.NSH, cfg.NW, cfg.NSW, cfg.WSZ
    H, HH, C, L, FPAD = cfg.H, cfg.HH, cfg.C, cfg.L, cfg.FPAD
    MCH, NQ = cfg.MCH, cfg.NQ
    NMC = NSH // MCH
    assert NMC * MCH == NSH
    CPQ = NQ // MCH
    assert CPQ * MCH == NQ
    IWALL = sum(IW)

    PCOL = {"encb": 0, "b2r": 1, "eps": 2, "tiny": 3, "zero": 4}
    nc_col = 5
    for l in range(L):
        for nm in ("g1", "be1", "b2c", "ng", "nb", "t"):
            PCOL[(nm, l)] = nc_col
            nc_col += 1
    NPCOL = nc_col

    nc = bacc.Bacc("TRN2", num_swdge_queues=4, dynamic_dma_scratch_size=32768)
    xt_d = nc.declare_dram_parameter("xt", [FPAD, NSH], dt.bfloat16, isOutput=False)
    encw_d = nc.declare_dram_parameter("encw", [128, FPAD // 128, H], dt.bfloat16, isOutput=False)
    w1_d = nc.declare_dram_parameter("w1", [H + 1, L, HH], dt.bfloat16, isOutput=False)
    w2_d = nc.declare_dram_parameter("w2", [HH, L, H], dt.bfloat16, isOutput=False)
    w2r_d = nc.declare_dram_parameter("w2r", [HH, H], dt.bfloat16, isOutput=False)
    linw_d = nc.declare_dram_parameter("linw", [H + 1, C], dt.bfloat16, isOutput=False)
    ident_d = nc.declare_dram_parameter("ident", [128, 128], dt.bfloat16, isOutput=False)
    params_d = nc.declare_dram_parameter("params", [128, NPCOL], dt.float32, isOutput=False)
    gidx_d = nc.declare_dram_parameter("gidx", [128, IWALL // 16], dt.int16, isOutput=False)
    rmat_d = nc.declare_dram_parameter("rmat", [128, CTOT, 128], dt.float8e4, isOutput=False)
    outp_d = nc.declare_dram_parameter("outp", [C, NSH], dt.float32, isOutput=True)

    uvshard = nc.dram_tensor("uvshard", [NSH, HH], dt.bfloat16)
    tabs = [nc.dram_tensor(f"uvtab{i}", [cfg.NPAD, HH], dt.bfloat16,
                           addr_space="Shared") for i in range(2)]

    with tile.TileContext(nc) as tc, ExitStack() as ctx:
        const = ctx.enter_context(tc.tile_pool(name="const", bufs=1))
        sb_par = const.tile([128, NPCOL], dt.float32)
        nc.sync.dma_start(sb_par[:], params_d[:])
        sb_encw = const.tile([128, FPAD // 128, H], dt.bfloat16)
        nc.sync.dma_start(sb_encw[:], encw_d[:])
        sb_w1 = const.tile([H + 1, L, HH], dt.bfloat16)
        nc.sync.dma_start(sb_w1[:], w1_d[:])
        sb_w2 = const.tile([HH, L, H], dt.bfloat16)
        nc.sync.dma_start(sb_w2[:], w2_d[:])
        sb_w2r = const.tile([HH, H], dt.bfloat16)
        nc.sync.dma_start(sb_w2r[:], w2r_d[:])
        sb_linw = const.tile([H + 1, C], dt.bfloat16)
        nc.sync.dma_start(sb_linw[:], linw_d[:])
        sb_id = const.tile([128, 128], dt.bfloat16)
        nc.sync.dma_start(sb_id[:], ident_d[:])
        sb_o128 = const.tile([128, 128], dt.bfloat16)
        nc.vector.memset(sb_o128[:], 1.0 / 128)
        sb_o64 = const.tile([H, H], dt.bfloat16)
        nc.vector.memset(sb_o64[:], 1.0 / H)
        sb_gidx = const.tile([128, IWALL // 16], dt.int16)
        nc.sync.dma_start(sb_gidx[:], gidx_d[:])

        def pcol(key, rows=128):
            cidx = PCOL[key]
            return sb_par[0:rows, cidx: cidx + 1]

        master = ctx.enter_context(tc.tile_pool(name="master", bufs=1))
        hT = master.tile([H + 1, NSH], dt.bfloat16)     # row H == 1.0
        numden = master.tile([HH, NSH], dt.bfloat16)
        uv2 = master.tile([128, NSH // 2], dt.bfloat16)
        rbf65 = master.tile([H + 1, NQ], dt.bfloat16)   # row H == 1.0
        nc.vector.memset(hT[H:H + 1, :], 1.0)
        nc.vector.memset(rbf65[H:H + 1, :], 1.0)

        # ---------------- encoder: hT = (x @ enc_W + b)^T ----------------
        with tc.tile_pool(name="enc", bufs=3) as ep, \
             tc.tile_pool(name="encps", bufs=2, space="PSUM") as pp:
            for c in range(NMC):
                sl = slice(c * MCH, (c + 1) * MCH)
                xtile = ep.tile([128, FPAD // 128, MCH], dt.bfloat16)
                for fc in range(FPAD // 128):
                    nc.sync.dma_start(
                        xtile[:, fc, :], xt_d[fc * 128:(fc + 1) * 128, sl])
                ps = pp.tile([H, MCH], dt.float32)
                for fc in range(FPAD // 128):
                    nc.tensor.matmul(ps[:], sb_encw[:, fc, :], xtile[:, fc, :],
                                     start=(fc == 0), stop=(fc == FPAD // 128 - 1))
                nc.vector.tensor_scalar_add(hT[0:H, sl], ps[:], pcol("encb", H))

        # persistent pools for the conv pipeline
        t64 = ctx.enter_context(tc.tile_pool(name="t64", bufs=4))
        t128 = ctx.enter_context(tc.tile_pool(name="t128", bufs=4))
        utp = ctx.enter_context(tc.tile_pool(name="utp", bufs=1))
        stp = ctx.enter_context(tc.tile_pool(name="stp", bufs=2))
        gp = ctx.enter_context(tc.tile_pool(name="gp", bufs=2))
        rp = ctx.enter_context(tc.tile_pool(name="rp", bufs=2))
        tpp = ctx.enter_context(tc.tile_pool(name="tpp", bufs=1, space="PSUM"))
        gsp = ctx.enter_context(tc.tile_pool(name="gsp", bufs=3, space="PSUM"))
        mpa = ctx.enter_context(tc.tile_pool(name="mpa", bufs=2, space="PSUM"))
        mpb = ctx.enter_context(tc.tile_pool(name="mpb", bufs=2, space="PSUM"))

        # ---------------- conv layers ----------------
        conv_params = [0] + list(range(L))          # [0, 0, 1, 2]
        for conv, l in enumerate(conv_params):
            is_first = conv == 0
            tab = tabs[conv % 2]

            # U phase: uv2 rows 0:H = u = m*exp(t*m), rows H:HH = w
            # (half-shard staging buffer; transpose each half then reuse)
            NWH = NW // 2
            for half in range(2):
                for qq in range(2):
                    q = half * 2 + qq
                    slq = slice(q * NQ, (q + 1) * NQ)
                    sll = slice(qq * NQ, (qq + 1) * NQ)
                    m_t = utp.tile([H, NQ], dt.bfloat16, tag="mt")
                    nc.vector.tensor_scalar(
                        m_t[:], hT[0:H, slq], 0.0, EPS_MSG,
                        AL.max, AL.add)
                    wq = utp.tile([H, NQ], dt.bfloat16, tag="wq")
                    nc.scalar.activation(wq[:], m_t[:], AF.Exp,
                                         scale=pcol(("t", l), H))
                    # partition-crossing copy (tensor_scalar base shift)
                    nc.vector.tensor_scalar_add(uv2[H:HH, sll], wq[:],
                                                pcol("zero", H))
                    nc.vector.tensor_mul(uv2[0:H, sll], m_t[:], wq[:])
                for ntl in range(NWH):
                    nt = half * NWH + ntl
                    tp = tpp.tile([128, HH], dt.bfloat16, tag="tp")
                    nc.tensor.transpose(
                        tp[:], uv2[:, ntl * 128:(ntl + 1) * 128], sb_id[:])
                    st = stp.tile([128, HH], dt.bfloat16, tag="st")
                    if nt % 2 == 0:
                        nc.scalar.copy(st[:], tp[:])
                    else:
                        nc.vector.tensor_copy(st[:], tp[:])
                    nc.sync.dma_start(uvshard[nt * 128:(nt + 1) * 128, :],
                                      st[:])
            nc.gpsimd.collective_compute(
                "AllGather", mybir.AluOpType.bypass,
                replica_groups=[list(range(NCORES))],
                ins=[uvshard[:, :]], outs=[tab[:, :]])

            # G+S: per window-group, 4 src-window sub-gathers (one per
            # SWDGE queue) + fp8 R matmuls accumulating all of a window's
            # chunks in PSUM, then one copy into numden
            for (w0, w1) in groups:
                slots_s = [int(PBc[w0:w1, si].sum()) for si in range(NSW)]
                tot = sum(slots_s)
                gbuf = gp.tile([128, tot, HH], dt.bfloat16, tag="gbuf")
                rtile = rp.tile([128, tot, 128], dt.float8e4, tag="rt")
                off = 0
                reg_off = []
                for si in range(NSW):
                    reg_off.append(off)
                    nsl = slots_s[si]
                    if nsl:
                        a = int(IW_base[si] + gx_off[w0, si])
                        nidx = nsl * 128
                        nc.gpsimd.dma_gather(
                            gbuf[:, off: off + nsl, :],
                            tab[si * WSZ: (si + 1) * WSZ, :],
                            sb_gidx[:, a // 16: (a + nidx) // 16],
                            nidx, nidx, HH, single_packet=False,
                            queue_num=si)
                        nc.sync.dma_start(
                            rtile[:, off: off + nsl, :],
                            rmat_d[:, a // 128: a // 128 + nsl, :])
                    off += nsl
                for wi in range(w0, w1):
                    nchw = int(PBc[wi].sum())
                    ps = gsp.tile([HH, 128], dt.float32, tag="gs")
                    done = 0
                    for si in range(NSW):
                        base = (reg_off[si]
                                + int(gx_off[wi, si] - gx_off[w0, si]) // 128)
                        for j in range(int(PBc[wi, si])):
                            nc.tensor.matmul(
                                ps[:], gbuf[:, base + j, :],
                                rtile[:, base + j, :],
                                start=(done == 0), stop=(done == nchw - 1))
                            done += 1
                    wsl = slice(wi * 128, (wi + 1) * 128)
                    if wi % 2 == 0:
                        nc.scalar.copy(numden[:, wsl], ps[:])
                    else:
                        nc.vector.tensor_copy(numden[:, wsl], ps[:])

            # M phase: agg -> MLP(+folded LN) -> residual, per quarter shard
            for q in range(4):
                slq = slice(q * NQ, (q + 1) * NQ)
                dn_lo = t64.tile([H, NQ], dt.bfloat16, tag="t64")
                nc.vector.tensor_scalar_max(dn_lo[:], numden[H:HH, slq], 0.5)
                s_t = t64.tile([H, NQ], dt.bfloat16, tag="t64")
                nc.scalar.activation(s_t[:], dn_lo[:],
                                     AF.Abs_reciprocal_sqrt,
                                     bias=pcol("tiny", H))
                rden = t64.tile([H, NQ], dt.bfloat16, tag="t64")
                nc.vector.tensor_mul(rden[:], s_t[:], s_t[:])
                t1 = t64.tile([H, NQ], dt.bfloat16, tag="t64")
                nc.vector.tensor_mul(t1[:], numden[0:H, slq], rden[:])
                nc.vector.tensor_add(rbf65[0:H, :], t1[:], hT[0:H, slq])
                ycs = t128.tile([HH, NQ], dt.bfloat16, tag="t128")
                rstd = t128.tile([HH, NQ], dt.bfloat16, tag="t128")
                for c in range(CPQ):
                    lo = c * MCH
                    ps1 = mpa.tile([HH, MCH], dt.float32, tag="mma")
                    nc.tensor.matmul(ps1[:], sb_w1[:, l, :],
                                     rbf65[:, lo:lo + MCH])
                    nc.vector.tensor_copy(ycs[:, lo:lo + MCH], ps1[:])
                sq = t128.tile([HH, NQ], dt.bfloat16, tag="t128")
                nc.vector.tensor_mul(sq[:], ycs[:], ycs[:])
                for c in range(CPQ):
                    lo = c * MCH
                    pv = mpa.tile([HH, MCH], dt.float32, tag="mma")
                    nc.tensor.matmul(pv[:], sb_o128[:], sq[:, lo:lo + MCH])
                    nc.scalar.activation(rstd[:, lo:lo + MCH], pv[:],
                                         AF.Abs_reciprocal_sqrt,
                                         bias=pcol("eps"))
                t_t = t128.tile([HH, NQ], dt.bfloat16, tag="t128")
                nc.vector.tensor_mul(t_t[:], ycs[:], rstd[:])
                h1 = t128.tile([HH, NQ], dt.bfloat16, tag="t128")
                nc.scalar.activation(h1[:], t_t[:], AF.Relu,
                                     bias=pcol(("be1", l)),
                                     scale=pcol(("g1", l)))
                if is_first:
                    for c in range(CPQ):
                        lo = c * MCH
                        sl = slice(q * NQ + lo, q * NQ + lo + MCH)
                        ps2 = mpb.tile([H, MCH], dt.float32, tag="mmb")
                        nc.tensor.matmul(ps2[:], sb_w2r[:, :], h1[:, lo:lo + MCH])
                        nc.vector.tensor_scalar_add(hT[0:H, sl], ps2[:],
                                                    pcol("b2r", H))
                else:
                    ycs2 = t64.tile([H, NQ], dt.bfloat16, tag="t64")
                    rstd2 = t64.tile([H, NQ], dt.bfloat16, tag="t64")
                    for c in range(CPQ):
                        lo = c * MCH
                        ps2 = mpb.tile([H, MCH], dt.float32, tag="mmb")
                        nc.tensor.matmul(ps2[:], sb_w2[:, l, :], h1[:, lo:lo + MCH])
                        nc.vector.tensor_scalar_add(ycs2[:, lo:lo + MCH], ps2[:],
                                                    pcol(("b2c", l), H))
                    sq2 = t64.tile([H, NQ], dt.bfloat16, tag="t64")
                    nc.vector.tensor_mul(sq2[:], ycs2[:], ycs2[:])
                    for c in range(CPQ):
                        lo = c * MCH
                        pv2 = mpb.tile([H, MCH], dt.float32, tag="mmb")
                        nc.tensor.matmul(pv2[:], sb_o64[:], sq2[:, lo:lo + MCH])
                        nc.scalar.activation(rstd2[:, lo:lo + MCH], pv2[:],
                                             AF.Abs_reciprocal_sqrt,
                                             bias=pcol("eps", H))
                    t2 = t64.tile([H, NQ], dt.bfloat16, tag="t64")
                    nc.vector.tensor_mul(t2[:], ycs2[:], rstd2[:])
                    c_t = t64.tile([H, NQ], dt.bfloat16, tag="t64")
                    nc.scalar.activation(c_t[:], t2[:], AF.Relu,
                                         bias=pcol(("nb", l), H),
                                         scale=pcol(("ng", l), H))
                    nc.vector.tensor_add(hT[0:H, slq], hT[0:H, slq], c_t[:])

        # ---------------- final head (reuses conv pools) ----------------
        for q in range(4):
            slq = slice(q * NQ, (q + 1) * NQ)
            yc = t128.tile([H, NQ], dt.bfloat16, tag="t128")
            rstd = t64.tile([H, NQ], dt.bfloat16, tag="t64")
            for c in range(CPQ):
                lo = c * MCH
                sl = slice(q * NQ + lo, q * NQ + lo + MCH)
                pmu = mpb.tile([H, MCH], dt.float32, tag="mmb")
                nc.tensor.matmul(pmu[:], sb_o64[:], hT[0:H, sl])
                nc.vector.tensor_sub(yc[:, lo:lo + MCH], hT[0:H, sl], pmu[:])
            sq = t64.tile([H, NQ], dt.bfloat16, tag="t64")
            nc.vector.tensor_mul(sq[:], yc[:], yc[:])
            for c in range(CPQ):
                lo = c * MCH
                pv = mpb.tile([H, MCH], dt.float32, tag="mmb")
                nc.tensor.matmul(pv[:], sb_o64[:], sq[:, lo:lo + MCH])
                nc.scalar.activation(rstd[:, lo:lo + MCH], pv[:],
                                     AF.Abs_reciprocal_sqrt, bias=pcol("eps", H))
            t_t = t64.tile([H, NQ], dt.bfloat16, tag="t64")
            nc.vector.tensor_mul(t_t[:], yc[:], rstd[:])
            f65 = t128.tile([H + 1, NQ], dt.bfloat16, tag="t128")
            nc.vector.memset(f65[H:H + 1, :], 1.0)
            nc.scalar.activation(f65[0:H, :], t_t[:], AF.Relu,
                                 bias=pcol(("nb", 0), H),
                                 scale=pcol(("ng", 0), H))
            for c in range(CPQ):
                lo = c * MCH
                sl = slice(q * NQ + lo, q * NQ + lo + MCH)
                pso = mpa.tile([C, MCH], dt.float32, tag="mma")
                nc.tensor.matmul(pso[:], sb_linw[:, :], f65[:, lo:lo + MCH])
                ot = stp.tile([C, MCH], dt.float32, tag="ot")
                nc.vector.tensor_copy(ot[:], pso[:])
                nc.sync.dma_start(outp_d[:, sl], ot[:])

    nc.compile()
    return nc, NPCOL, PCOL


# --------------------------------------------------------------------------
# host: input packing
# --------------------------------------------------------------------------

def pack_inputs(inputs, structs, NPCOL, PCOL, cfg=CFG):
    bf16 = ml_dtypes.bfloat16
    NSH, NPAD, FPAD = cfg.NSH, cfg.NPAD, cfg.FPAD
    H, HH, C, L = cfg.H, cfg.HH, cfg.C, cfg.L

    x = np.asarray(inputs["x"], np.float32)
    xp = np.zeros((NPAD, FPAD), np.float32)
    xp[: x.shape[0], : x.shape[1]] = x

    encw = np.zeros((FPAD, H), np.float32)
    encw[: cfg.F_IN] = np.asarray(inputs["enc_W"], np.float32)
    encw = np.ascontiguousarray(
        encw.reshape(FPAD // 128, 128, H).transpose(1, 0, 2)).astype(bf16)

    # W1 with LN mean-centering folded in + centered bias as a 65th row
    W1 = np.asarray(inputs["W1"], np.float32)          # [L, H, HH]
    b1 = np.asarray(inputs["b1"], np.float32)          # [L, HH]
    W1c = W1 - W1.mean(axis=2, keepdims=True)
    b1c = b1 - b1.mean(axis=1, keepdims=True)
    w1s = np.concatenate([W1c, b1c[:, None, :]], axis=1)   # [L, H+1, HH]
    w1s = np.ascontiguousarray(w1s.transpose(1, 0, 2)).astype(bf16)

    W2 = np.asarray(inputs["W2"], np.float32)          # [L, HH, H]
    W2c = W2 - W2.mean(axis=2, keepdims=True)
    w2s = np.ascontiguousarray(W2c.transpose(1, 0, 2)).astype(bf16)
    w2r = np.ascontiguousarray(W2[0]).astype(bf16)     # uncentered, conv 0

    linw = np.asarray(inputs["lin_W"], np.float32)
    linb = np.asarray(inputs["lin_b"], np.float32)
    linw65 = np.concatenate([linw, linb[None, :]], axis=0).astype(bf16)
    ident = np.eye(128, dtype=bf16)

    b2 = np.asarray(inputs["b2"], np.float32)          # [L, H]
    b2c = b2 - b2.mean(axis=1, keepdims=True)

    params = np.zeros((128, NPCOL), np.float32)
    params[:H, PCOL["encb"]] = inputs["enc_b"]
    params[:H, PCOL["b2r"]] = b2[0]
    params[:, PCOL["eps"]] = LN_EPS
    params[:, PCOL["tiny"]] = 1e-30
    for l in range(L):
        params[:, PCOL[("g1", l)]] = inputs["g1"][l]
        params[:, PCOL[("be1", l)]] = inputs["be1"][l]
        params[:H, PCOL[("b2c", l)]] = b2c[l]
        params[:H, PCOL[("ng", l)]] = inputs["ng"][l]
        params[:H, PCOL[("nb", l)]] = inputs["nb"][l]
        params[:, PCOL[("t", l)]] = float(np.asarray(inputs["t"][l]))

    in_maps = []
    for k in range(NCORES):
        xs = np.ascontiguousarray(
            xp[k * NSH:(k + 1) * NSH].T).astype(bf16)
        gi = structs["gidx"][k]
        gw = np.tile(np.ascontiguousarray(gi.reshape(-1, 16).T), (8, 1))
        in_maps.append({
            "xt": xs, "encw": encw, "w1": w1s, "w2": w2s, "w2r": w2r,
            "linw": linw65, "ident": ident, "params": params, "gidx": gw,
            "rmat": structs["rmat"][k],
        })
    return in_maps


def _run(inputs, cfg=CFG, trace=False, tmpdir=None):
    import sys
    sys.path.insert(0, "/root/problem")
    from concourse.bass_utils import run_bass_kernel_spmd

    structs = build_edge_structs(inputs["edge_index"], cfg)
    nc, NPCOL, PCOL = build_nc(structs, cfg)
    in_maps = pack_inputs(inputs, structs, NPCOL, PCOL, cfg)
    res = run_bass_kernel_spmd(nc, in_maps, list(range(NCORES)), trace=trace,
                               tmpdir=tmpdir)
    outs = [res.results[k]["outp"] for k in range(NCORES)]  # [C, NSH] each
    full = np.concatenate(outs, axis=1).T                   # [NPAD, C]
    return np.ascontiguousarray(full[: cfg.N]).astype(np.float32), res


def kernel(**inputs) -> np.ndarray:
    out, _ = _run(inputs)
    return out
